# revision 1
# baseline (speedup 1.0000x reference)
"""Trainium2 Bass kernel for a pre-LN transformer block (B=2, S=2048, D=1024,
H=16, d_ff=4096), 8-way (batch, head-group) tensor-parallel:

- core c handles batch c//4 and heads 4*(c%4)..4*(c%4)+3: LN1+qkv run over the
  core's 2048 batch tokens only, attention over 4 heads
- softmax exp is split across engines: even key-tiles use the Activation
  engine's exact Exp, odd key-tiles use a Schraudolph-style int16 exponent
  construction on the DVE (bitcast to bf16)
- attention-proj partials are ReduceScattered in 4 feature chunks within each
  4-core batch group (two groups run concurrently), overlapping the proj tail
  and the LN2 residual/stats pipeline
- token-sharded MLP with the full d_ff on each core (no second collective)

Activations live feature-major [feature, token].  LayerNorm is folded into the
matmuls via an augmented contraction row (-mu) and column (row-sums of the
g-scaled weights); the 1/sigma factor is applied on PSUM eviction.  Softmax is
computed unnormalized with a ones-column appended to V producing row sums, and
1/sum is applied on the attention-output eviction.
"""

import sys

for _p in ("/opt/trn_rl_repo",):
    if _p not in sys.path:
        sys.path.insert(0, _p)

import numpy as np
import ml_dtypes

B, S, D = 2, 2048, 1024
H, HD = 16, 64
FF = 4 * D
T = B * S  # 4096 tokens
NCORES = 8
TC = T // NCORES  # 512 tokens per core (MLP/out shard)
TB = S  # 2048 tokens per batch (per-core attention range)
P = 128
KT = D // P  # 8 k-tiles over D
KA = 9  # augmented k-tiles
DAUG = D + P  # 1152
EPS = 1e-5
NKT = TB // P  # 16 key tiles per batch
NQC = TB // 512  # 4 q-chunks of 512
BF16 = ml_dtypes.bfloat16

# Schraudolph exp: bf16 bits ~= round(x*log2(e)*128 + (127*128 - 7.63))
LOG2E = float(np.log2(np.e))
EXP_A = 128.0 * LOG2E / np.sqrt(HD)  # logit scale 1/sqrt(HD) folded in
EXP_B = 127.0 * 128.0 - 7.63
# key tiles using exact Exp on the Activation engine (rest: Schraudolph on DVE)
SC_KT = frozenset({0, 2, 4, 6, 8, 10, 12, 14})

_CACHE = {}


def _build_program(has_c1, has_bproj, has_c2, has_b1, has_b2):
    import concourse.mybir as mybir
    import concourse.tile as tile
    from concourse import bacc
    from concourse.masks import make_identity
    from contextlib import ExitStack

    f32 = mybir.dt.float32
    bf16 = mybir.dt.bfloat16
    i16 = mybir.dt.int16
    AF = mybir.ActivationFunctionType
    ALU = mybir.AluOpType

    nc = bacc.Bacc(None, target_bir_lowering=False)

    # ---- I/O ----
    x_aug_d = nc.declare_dram_parameter("x_aug", [DAUG, TB], bf16, isOutput=False)
    x_c_d = nc.declare_dram_parameter("x_c", [D, TC], f32, isOutput=False)
    wqkv_d = nc.declare_dram_parameter("wqkv_aug", [DAUG, 6 * P], bf16, isOutput=False)
    wproj_d = nc.declare_dram_parameter("wproj_c", [2 * P, D], bf16, isOutput=False)
    w1_d = nc.declare_dram_parameter("w1_aug", [D, FF], bf16, isOutput=False)
    w2t_d = nc.declare_dram_parameter("w2t", [FF, D], bf16, isOutput=False)
    aux_d = nc.declare_dram_parameter("aux", [P, 64], f32, isOutput=False)
    # aux columns: 0:8 -> b_proj as [128,8], 8:40 -> b1 as [128,32],
    # 40:48 -> b2 as [128,8], 48:54 -> C1 (qkv bias-fold) as [128,6]
    out_d = nc.declare_dram_parameter("out_c", [D, TC], f32, isOutput=True)

    groups = [[0, 1, 2, 3], [4, 5, 6, 7]]

    with tile.TileContext(nc) as tc, ExitStack() as ctx:
        const = ctx.enter_context(tc.tile_pool(name="const", bufs=1))
        dram = ctx.enter_context(tc.tile_pool(name="dram", bufs=1, space="DRAM"))

        ident = const.tile([P, P], bf16)
        make_identity(nc, ident)
        ones128 = const.tile([P, P], bf16)
        nc.any.memset(ones128, 1.0)
        eps_col = const.tile([P, 1], f32)
        nc.any.memset(eps_col, EPS)

        wqkv_sb = const.tile([P, KA, 6 * P], bf16)
        nc.sync.dma_start(wqkv_sb, wqkv_d.rearrange("(k p) e -> p k e", p=P))
        wproj_sb = const.tile([P, 2, D], bf16)
        nc.sync.dma_start(wproj_sb, wproj_d.rearrange("(k p) d -> p k d", p=P))
        aux_sb = const.tile([P, 64], f32)
        nc.sync.dma_start(aux_sb, aux_d[:])

        # long-lived activation tensors
        x1grp = ctx.enter_context(tc.tile_pool(name="x1grp", bufs=1))
        x1 = x1grp.tile([P, KT, TC], f32)
        x1aug = x1grp.tile([P, KT, TC], bf16)
        h2T = ctx.enter_context(tc.tile_pool(name="h2", bufs=1)).tile(
            [P, FF // P, TC], bf16
        )
        work = ctx.enter_context(tc.tile_pool(name="work", bufs=2))

        psA = ctx.enter_context(tc.tile_pool(name="psA", bufs=2, space="PSUM"))

        # partial proj sums, 2 feature-quad chunks: rows = tch*512 + (m%4)*128
        partial_j = [
            dram.tile([4 * 4 * P, TC], bf16, tag=f"pj{j}", name=f"pj{j}")
            for j in range(2)
        ]
        x1proj_d = dram.tile([D, TC], bf16, tag="x1proj", name="x1proj")

        x_aug_r = x_aug_d.rearrange("(k p) t -> p k t", p=P)
        w1_noaug_r = w1_d.rearrange("(k p) f -> p k f", p=P)

        with tc.tile_pool(name="qkvTp", bufs=1) as qkvT_pool, \
             tc.tile_pool(name="attnTp", bufs=1) as attnT_pool, \
             tc.tile_pool(name="attg", bufs=1) as attg, \
             tc.tile_pool(name="etp", bufs=8) as etp, \
             tc.tile_pool(name="poutp", bufs=4) as poutp, \
             tc.tile_pool(name="lgp", bufs=3, space="PSUM") as lgp, \
             tc.tile_pool(name="avqp", bufs=2, space="PSUM") as avqp, \
             tc.tile_pool(name="rbpp", bufs=1, space="PSUM") as rbpp:
            qkvT = [qkvT_pool.tile([P, 3, TB], bf16, name=f"qkvT{pt}") for pt in (0, 1)]
            attnT = [attnT_pool.tile([P, TB], bf16, name=f"attnT{pt}") for pt in (0, 1)]
            # vext: per key tile: [h0 | 1 | h1 | 1 | h2 | 1 | h3 | 1]
            vext = attg.tile([P, NKT, 4 * 65], bf16)

            # ============ phase A: LN1 stats + qkv + vext, per token chunk ===
            with tc.tile_pool(name="xaug", bufs=2) as xaug_pool, \
                 nc.named_scope("ln1_qkv"):
                for hp in range(4):
                    nc.any.memset(vext[:, :, hp * 65 + 64 : hp * 65 + 65], 1.0)
                for tch in range(NQC):
                    tsl = slice(tch * 512, (tch + 1) * 512)
                    xa = xaug_pool.tile([P, KA, 512], bf16, tag="xa")
                    nc.sync.dma_start(xa, x_aug_r[:, :, tsl])
                    pmu = psA.tile([P, 512], f32, tag="a", name="pmu")
                    psq = psA.tile([P, 512], f32, tag="a", name="psq")
                    for kt in range(KT):
                        xsq = work.tile([P, 512], bf16, tag="xsq")
                        nc.vector.tensor_tensor(
                            xsq, xa[:, kt, :], xa[:, kt, :], ALU.mult
                        )
                        nc.tensor.matmul(
                            pmu, ones128, xa[:, kt, :],
                            start=(kt == 0), stop=(kt == KT - 1),
                        )
                        nc.tensor.matmul(
                            psq, ones128, xsq,
                            start=(kt == 0), stop=(kt == KT - 1),
                        )
                    m1 = work.tile([P, 512], f32, tag="m1")
                    nc.vector.tensor_scalar_mul(m1, pmu, 1.0 / D)
                    # augmented row: -mu (bf16), partition 0 of k-tile 8
                    nc.vector.tensor_scalar_mul(xa[0:1, KT, :], m1[0:1, :], -1.0)
                    v1 = work.tile([P, 512], f32, tag="v1")
                    nc.vector.tensor_scalar_mul(v1, psq, 1.0 / D)
                    m2 = work.tile([P, 512], f32, tag="m2")
                    nc.vector.tensor_tensor(m2, m1, m1, ALU.mult)
                    nc.vector.tensor_tensor(v1, v1, m2, ALU.subtract)
                    sd = work.tile([P, 512], f32, tag="sd")
                    nc.scalar.activation(sd, v1, AF.Sqrt, bias=eps_col)
                    r1b = xaug_pool.tile([P, 512], f32, tag="r1b")
                    nc.vector.reciprocal_approx_fast(r1b, sd)

                    for pt in range(2):
                        for m in range(3):
                            msl = slice(pt * 3 * P + m * P, pt * 3 * P + (m + 1) * P)
                            ps = lgp.tile([P, 512], f32, tag="lg", name="qkvps")
                            for kt in range(KA):
                                nc.tensor.matmul(
                                    ps, wqkv_sb[:, kt, msl], xa[:, kt, :],
                                    start=(kt == 0), stop=(kt == KA - 1),
                                )
                            dst = qkvT[pt][:, m, tsl]
                            nc.vector.tensor_tensor(dst, ps, r1b, ALU.mult)
                            if has_c1:
                                nc.vector.tensor_scalar(
                                    dst, dst,
                                    aux_sb[:, 48 + pt * 3 + m : 49 + pt * 3 + m],
                                    None, ALU.add,
                                )
                    # vext for this chunk's 4 key tiles
                    with nc.named_scope("vext"):
                        for k4 in range(4):
                            kt = tch * 4 + k4
                            ksl = slice(kt * P, (kt + 1) * P)
                            for pt in range(2):
                                pt_t = psA.tile([P, 512], bf16, tag="a", name="ptt")[
                                    :, 0:P
                                ]
                                nc.tensor.transpose(pt_t, qkvT[pt][:, 2, ksl], ident)
                                c0 = pt * 130
                                nc.vector.tensor_copy(
                                    vext[:, kt, c0 : c0 + 64], pt_t[:, 0:64]
                                )
                                nc.vector.tensor_copy(
                                    vext[:, kt, c0 + 65 : c0 + 129], pt_t[:, 64:128]
                                )

            # ============ phase B: attention ================================
            with nc.named_scope("attn"):
                from collections import deque

                epi_q = deque()
                epi_b = deque()
                proj_q = deque()
                rs_fired = [False] * 2

                def _epilogue_a(st):
                    pt, hp, qc, avq = st
                    rs_sb = attg.tile([1, 512], f32, tag="rsb", name="rs_sb", bufs=2)
                    nc.scalar.activation(rs_sb, avq[64:65, :], AF.Copy)
                    rc_f = attg.tile([1, 512], f32, tag="rcf", name="rcf", bufs=2)
                    nc.vector.reciprocal_approx_fast(rc_f, rs_sb)
                    rc_b = attg.tile([1, 512], bf16, tag="rcb", name="rcb", bufs=2)
                    nc.scalar.activation(rc_b, rc_f, AF.Copy)
                    return (pt, hp, qc, avq, rc_b)

                def _epilogue_b(st):
                    pt, hp, qc, avq, rc_b = st
                    q0 = qc * 512
                    rbp = rbpp.tile([P, 512], f32, tag="rb", name="rbp")[0:64, :]
                    nc.tensor.matmul(
                        rbp, ones128[0:1, 0:64], rc_b, start=True, stop=True
                    )
                    rbs = attg.tile([64, 512], bf16, tag="rbs", name="rbs", bufs=2)
                    nc.scalar.activation(rbs, rbp, AF.Copy)
                    nc.vector.tensor_tensor(
                        attnT[pt][hp * HD : (hp + 1) * HD, q0 : q0 + 512],
                        avq[0:64, :], rbs, ALU.mult,
                    )

                def _emit_proj():
                    qc, m = proj_q.popleft()
                    tsl = slice(qc * 512, (qc + 1) * 512)
                    ps = psA.tile([P, 512], f32, tag="a", name="projps")
                    for kt2 in range(2):
                        nc.tensor.matmul(
                            ps, wproj_sb[:, kt2, m * P : (m + 1) * P],
                            attnT[kt2][:, tsl], start=(kt2 == 0), stop=(kt2 == 1),
                        )
                    pb = poutp.tile([P, TC], bf16, tag="pout", name="pb")
                    nc.scalar.activation(pb, ps, AF.Copy)
                    j = m // 4
                    nc.sync.dma_start(
                        partial_j[j][qc * 4 * P + (m % 4) * P :
                                     qc * 4 * P + (m % 4 + 1) * P, :],
                        pb,
                    )
                    if qc == NQC - 1 and m % 4 == 3 and not rs_fired[j]:
                        rs_fired[j] = True
                        with nc.named_scope("reducescatter"):
                            nc.gpsimd.collective_compute(
                                "ReduceScatter",
                                mybir.AluOpType.add,
                                replica_groups=groups,
                                ins=[partial_j[j][:]],
                                outs=[x1proj_d[j * 4 * P : (j + 1) * 4 * P, :]],
                            )

                for qc in range(NQC):
                    for u in range(4):
                        pt, hp = u // 2, u % 2
                        hsl = slice(hp * HD, (hp + 1) * HD)
                        q0 = qc * 512
                        avq = avqp.tile([P, 512], f32, tag="avq",
                                        name=f"avq{qc}{u}")[0:65, :]
                        for kt in range(NKT):
                            ksl = slice(kt * P, (kt + 1) * P)
                            lg = lgp.tile([P, 512], f32, tag="lg", name="lg")
                            nc.tensor.matmul(
                                lg, qkvT[pt][hsl, 1, ksl],
                                qkvT[pt][hsl, 0, q0 : q0 + 512],
                                start=True, stop=True,
                            )
                            et = etp.tile([P, 512], bf16, tag="et")
                            if kt in SC_KT:
                                nc.scalar.activation(
                                    et, lg, AF.Exp, scale=1.0 / np.sqrt(HD)
                                )
                            else:
                                nc.vector.tensor_scalar(
                                    et.bitcast(i16), lg, EXP_A, EXP_B,
                                    ALU.mult, ALU.add,
                                )
                            vcol = slice(pt * 130 + hp * 65, pt * 130 + hp * 65 + 65)
                            nc.tensor.matmul(
                                avq, vext[:, kt, vcol], et,
                                start=(kt == 0), stop=(kt == NKT - 1),
                            )
                            if kt == 1 and epi_q:
                                epi_b.append(_epilogue_a(epi_q.popleft()))
                            if kt == 6 and epi_b:
                                _epilogue_b(epi_b.popleft())
                            if u > 0 and kt in (3, 9, 13) and proj_q:
                                _emit_proj()
                        epi_q.append((pt, hp, qc, avq))
                    proj_q.extend((qc, m) for m in range(8))

                while epi_q:
                    epi_b.append(_epilogue_a(epi_q.popleft()))
                while epi_b:
                    _epilogue_b(epi_b.popleft())
                with nc.named_scope("proj"):
                    while proj_q:
                        _emit_proj()

        w_stack = ExitStack()
        w_pool = w_stack.enter_context(tc.tile_pool(name="wpool", bufs=1))
        psB = w_stack.enter_context(tc.tile_pool(name="psB", bufs=3, space="PSUM"))
        FQ = FF // 4
        w1q = []
        for q in range(4):
            w1qt = w_pool.tile([P, KT, FQ], bf16, tag=f"w{q}", name=f"w1q{q}")
            nc.sync.dma_start(w1qt, w1_noaug_r[:, :, q * FQ : (q + 1) * FQ])
            w1q.append(w1qt)

        # ============ phase C: residual + LN2, pipelined per RS chunk =====
        with tc.tile_pool(name="resid", bufs=1) as resid, \
             tc.tile_pool(name="x1pp", bufs=2) as x1pp, nc.named_scope("x1_ln2"):
            xc = resid.tile([P, KT, TC], f32, tag="xc")
            nc.sync.dma_start(xc, x_c_d.rearrange("(k p) t -> p k t", p=P))
            xb = resid.tile([P, KT, TC], bf16, tag="xb")
            pmu = psA.tile([P, 512], f32, tag="a", name="pmu2")
            psq = psA.tile([P, 512], f32, tag="a", name="psq2")
            x1p_r = x1proj_d.rearrange("(k p) t -> p k t", p=P)
            for g in range(4):
                x1p = x1pp.tile([P, 2, TC], bf16, tag="x1p")
                nc.sync.dma_start(x1p, x1p_r[:, 2 * g : 2 * g + 2, :])
                for sub in range(2):
                    kt = 2 * g + sub
                    nc.vector.tensor_tensor(
                        x1[:, kt, :], xc[:, kt, :], x1p[:, sub, :], ALU.add
                    )
                    if has_bproj:
                        nc.vector.tensor_scalar(
                            x1[:, kt, :], x1[:, kt, :],
                            aux_sb[:, kt : kt + 1], None, ALU.add,
                        )
                    nc.vector.tensor_copy(xb[:, kt, :], x1[:, kt, :])
                    xsq = work.tile([P, TC], bf16, tag="xsq")
                    nc.vector.tensor_tensor(
                        xsq, xb[:, kt, :], xb[:, kt, :], ALU.mult
                    )
                    nc.tensor.matmul(
                        pmu, ones128, xb[:, kt, :],
                        start=(kt == 0), stop=(kt == KT - 1),
                    )
                    nc.tensor.matmul(
                        psq, ones128, xsq, start=(kt == 0), stop=(kt == KT - 1)
                    )
            m1 = work.tile([P, TC], f32, tag="m1")
            nc.vector.tensor_scalar_mul(m1, pmu, 1.0 / D)
            v1 = work.tile([P, TC], f32, tag="v1")
            nc.vector.tensor_scalar_mul(v1, psq, 1.0 / D)
            m2 = work.tile([P, TC], f32, tag="m2")
            nc.vector.tensor_tensor(m2, m1, m1, ALU.mult)
            nc.vector.tensor_tensor(v1, v1, m2, ALU.subtract)
            sd = work.tile([P, TC], f32, tag="sd")
            nc.scalar.activation(sd, v1, AF.Sqrt, bias=eps_col)
            r2b = work.tile([P, TC], f32, tag="r2b")
            nc.vector.reciprocal_approx_fast(r2b, sd)
            m1b = work.tile([P, TC], bf16, tag="m1b")
            nc.scalar.activation(m1b, m1, AF.Copy)
            r2s = work.tile([P, TC], bf16, tag="r2s")
            nc.scalar.activation(r2s, r2b, AF.Copy)
            # x1aug = (x1 - mu)/sigma in bf16 (all-bf16 operands for DVE 2x/4x)
            for kt in range(KT):
                nc.vector.tensor_tensor(
                    x1aug[:, kt, :], xb[:, kt, :], m1b, ALU.subtract
                )
                nc.vector.tensor_tensor(
                    x1aug[:, kt, :], x1aug[:, kt, :], r2s, ALU.mult
                )

        # ============ phase D: MLP up, MLP down trailing by half ==========
        NF = FF // P  # 32 f-tiles
        NQ = NF // 4  # 8 f-tiles per weight quarter
        w2r = w2t_d.rearrange("(k p) d -> p k d", p=P)
        w2q = [None] * 4

        def _w2s(kt, m):
            return w2q[kt // NQ][:, kt % NQ, m * P : (m + 1) * P]

        accs3 = [
            psB.tile([P, 1024], f32, tag="b", name=f"m2p{g}") for g in range(3)
        ]
        accs = [accs3[m // 2][:, (m % 2) * TC : (m % 2 + 1) * TC] for m in range(6)]
        with nc.named_scope("mlp"):
            assert not has_c2, "nonzero ln2_b not supported"
            for j in range(NF):
                if j >= NQ and j % NQ == 0:
                    q = j // NQ - 1
                    w2q[q] = w_pool.tile(
                        [P, NQ, D], bf16, tag=f"w{q}", name=f"w2q{q}"
                    )
                    nc.sync.dma_start(w2q[q], w2r[:, q * NQ : (q + 1) * NQ, :])
                w1h = w1q[j // NQ]
                msl = slice((j % NQ) * P, (j % NQ + 1) * P)
                ps = psA.tile([P, 512], f32, tag="a", name="m1ps")
                for kt in range(KT):
                    nc.tensor.matmul(
                        ps, w1h[:, kt, msl], x1aug[:, kt, :],
                        start=(kt == 0), stop=(kt == KT - 1),
                    )
                bias_arg = aux_sb[:, 8 + j : 9 + j] if has_b1 else 0.0
                nc.scalar.activation(h2T[:, j, :], ps, AF.Relu, bias=bias_arg)
                if j >= 2 * NQ:
                    kt2 = j - 2 * NQ
                    for m in range(6):
                        nc.tensor.matmul(
                            accs[m], _w2s(kt2, m), h2T[:, kt2, :],
                            start=(kt2 == 0), stop=False,
                        )
            w2q[3] = w_pool.tile([P, NQ, D], bf16, tag="w3", name="w2q3")
            nc.sync.dma_start(w2q[3], w2r[:, 3 * NQ :, :])
            for kt2 in range(NF - 2 * NQ, NF):
                for m in range(6):
                    nc.tensor.matmul(
                        accs[m], _w2s(kt2, m), h2T[:, kt2, :],
                        start=False, stop=(kt2 == NF - 1),
                    )
            for m in range(6):
                acc = accs[m]
                ob = work.tile([P, TC], f32, tag="ob")
                nc.vector.tensor_tensor(ob, acc, x1[:, m, :], ALU.add)
                if has_b2:
                    nc.vector.tensor_scalar(
                        ob, ob, aux_sb[:, 40 + m : 41 + m], None, ALU.add
                    )
                nc.sync.dma_start(out_d[m * P : (m + 1) * P, :], ob)
            tails = [
                psA.tile([P, 512], f32, tag="a", name=f"m2t{m}") for m in range(2)
            ]
            for kt in range(NF):
                for m in range(2):
                    nc.tensor.matmul(
                        tails[m], _w2s(kt, 6 + m), h2T[:, kt, :],
                        start=(kt == 0), stop=(kt == NF - 1),
                    )
            for m in range(2):
                ob = work.tile([P, TC], f32, tag="ob")
                nc.vector.tensor_tensor(ob, tails[m], x1[:, 6 + m, :], ALU.add)
                if has_b2:
                    nc.vector.tensor_scalar(
                        ob, ob, aux_sb[:, 46 + m : 47 + m], None, ALU.add
                    )
                nc.sync.dma_start(out_d[(6 + m) * P : (7 + m) * P, :], ob)
        w_stack.close()

    nc.compile()
    return nc


def _prep_inputs(inputs):
    x = np.asarray(inputs["x"], np.float32)
    w_qkv = np.asarray(inputs["w_qkv"], np.float32)
    w_proj = np.asarray(inputs["w_proj"], np.float32)
    b_proj = np.asarray(inputs["b_proj"], np.float32)
    w1 = np.asarray(inputs["w1"], np.float32)
    b1 = np.asarray(inputs["b1"], np.float32)
    w2 = np.asarray(inputs["w2"], np.float32)
    b2 = np.asarray(inputs["b2"], np.float32)
    ln1_g = np.asarray(inputs["ln1_g"], np.float32)
    ln1_b = np.asarray(inputs["ln1_b"], np.float32)
    ln2_g = np.asarray(inputs["ln2_g"], np.float32)
    ln2_b = np.asarray(inputs["ln2_b"], np.float32)

    has_c1 = bool(np.any(ln1_b != 0))
    has_bproj = bool(np.any(b_proj != 0))
    has_c2 = bool(np.any(ln2_b != 0))
    has_b1 = bool(np.any(b1 != 0))
    has_b2 = bool(np.any(b2 != 0))
    flags = (has_c1, has_bproj, has_c2, has_b1, has_b2)

    xT = np.ascontiguousarray(x.reshape(T, D).T)  # [D, T] f32

    wg = w_qkv * ln1_g[None, :]  # [3D, D]
    Se = wg.sum(axis=1)  # [3D]
    Ce = w_qkv @ ln1_b  # [3D]
    w1g = w1 * ln2_g[None, :]  # [FF, D]
    C2 = w1 @ ln2_b
    if np.any(C2 != 0):
        raise NotImplementedError("nonzero ln2_b not supported")

    w1_aug = np.ascontiguousarray(w1g.T).astype(BF16)
    w2t = np.ascontiguousarray(w2.T).astype(BF16)  # [FF, D]

    in_maps = []
    for c in range(NCORES):
        bc, hg = c // 4, c % 4
        # batch-sliced augmented x
        x_aug = np.zeros((DAUG, TB), BF16)
        x_aug[:D] = xT[:, bc * TB : (bc + 1) * TB].astype(BF16)

        # qkv weights for 4 heads: two partition-tiles of head pairs
        wqkv_aug = np.zeros((DAUG, 6 * P), BF16)
        cstack = np.zeros((P, 6), np.float32)
        for pt in range(2):
            r0 = (4 * hg + 2 * pt) * HD  # 128 contiguous rows (2 heads)
            for m in range(3):
                rows = slice(m * D + r0, m * D + r0 + 2 * HD)
                csl = slice(pt * 3 * P + m * P, pt * 3 * P + (m + 1) * P)
                wqkv_aug[:D, csl] = wg[rows].T.astype(BF16)
                wqkv_aug[D, csl] = Se[rows].astype(BF16)
                cstack[:, pt * 3 + m] = Ce[rows]

        # proj rows for this core's 256 head dims
        wproj_c = np.ascontiguousarray(
            w_proj[:, 4 * hg * HD : (4 * hg + 4) * HD].T
        ).astype(BF16)  # [256, D]

        aux = np.zeros((P, 64), np.float32)
        aux[:, 0:8] = b_proj.reshape(KT, P).T
        aux[:, 8:40] = b1.reshape(FF // P, P).T
        aux[:, 40:48] = b2.reshape(KT, P).T
        aux[:, 48:54] = cstack

        in_maps.append(
            {
                "x_aug": x_aug,
                "x_c": np.ascontiguousarray(xT[:, c * TC : (c + 1) * TC]),
                "wqkv_aug": wqkv_aug,
                "wproj_c": wproj_c,
                "w1_aug": w1_aug,
                "w2t": w2t,
                "aux": aux,
            }
        )
    return flags, in_maps


def _run(inputs, trace=False, trace_kwargs=None):
    from concourse.bass_utils import run_bass_kernel_spmd

    flags, in_maps = _prep_inputs(inputs)
    if flags not in _CACHE:
        _CACHE[flags] = _build_program(*flags)
    nc = _CACHE[flags]
    res = run_bass_kernel_spmd(
        nc, in_maps, list(range(NCORES)), trace=trace,
        **(trace_kwargs or {}),
    )
    outT = np.empty((D, T), np.float32)
    for c in range(NCORES):
        outT[:, c * TC : (c + 1) * TC] = res.results[c]["out_c"]
    out = np.ascontiguousarray(outT.T).reshape(B, S, D)
    return out, res


def kernel(**inputs):
    out, _ = _run(inputs, trace=False)
    return out



# revision 26
# speedup vs baseline: 1.0314x; 1.0314x over previous
"""Trainium2 Bass kernel for a pre-LN transformer block (B=2, S=2048, D=1024,
H=16, d_ff=4096), 8-way (batch, head-group) tensor-parallel:

- core c handles batch c//4 and heads 4*(c%4)..4*(c%4)+3: LN1+qkv run over the
  core's 2048 batch tokens only, attention over 4 heads
- softmax exp is split across engines: even key-tiles use the Activation
  engine's exact Exp, odd key-tiles use a Schraudolph-style int16 exponent
  construction on the DVE (bitcast to bf16)
- attention-proj partials are ReduceScattered per query-chunk (4 collectives),
  each fired as soon as that chunk's proj partials are done, so 3 of 4 overlap
  the remaining attention compute; each core owns four interleaved 128-token
  slabs (slab qc = tokens qc*512 + rank*128 ..+128) so the residual+LN2+MLP
  pipeline starts at attention end, with the MLP split into two 256-token
  passes (the second gated only on the last collective)
- token-sharded MLP with the full d_ff on each core (no second collective)

Activations live feature-major [feature, token].  LayerNorm is folded into the
matmuls via an augmented contraction row (-mu) and column (row-sums of the
g-scaled weights); the 1/sigma factor is applied on PSUM eviction.  Softmax is
computed unnormalized with a ones-column appended to V producing row sums, and
1/sum is applied on the attention-output eviction.
"""

import sys

for _p in ("/opt/trn_rl_repo",):
    if _p not in sys.path:
        sys.path.insert(0, _p)

import numpy as np
import ml_dtypes

B, S, D = 2, 2048, 1024
H, HD = 16, 64
FF = 4 * D
T = B * S  # 4096 tokens
NCORES = 8
TC = T // NCORES  # 512 tokens per core (MLP/out shard)
TB = S  # 2048 tokens per batch (per-core attention range)
P = 128
KT = D // P  # 8 k-tiles over D
KA = 9  # augmented k-tiles
DAUG = D + P  # 1152
EPS = 1e-5
NKT = TB // P  # 16 key tiles per batch
NQC = TB // 512  # 4 q-chunks of 512
SLAB = TC // NQC  # 128 tokens per owned slab
BF16 = ml_dtypes.bfloat16

# Schraudolph exp: bf16 bits ~= round(x*log2(e)*128 + (127*128 - 7.63))
LOG2E = float(np.log2(np.e))
EXP_A = 128.0 * LOG2E / np.sqrt(HD)  # logit scale 1/sqrt(HD) folded in
EXP_B = 127.0 * 128.0 - 7.63
# key tiles using exact Exp on the Activation engine (rest: Schraudolph on DVE)
SC_KT = frozenset({0, 2, 4, 6, 8, 10, 12, 14})

_CACHE = {}


def _build_program(has_c1, has_bproj, has_c2, has_b1, has_b2):
    import concourse.mybir as mybir
    import concourse.tile as tile
    from concourse import bacc
    from concourse.masks import make_identity
    from contextlib import ExitStack

    f32 = mybir.dt.float32
    bf16 = mybir.dt.bfloat16
    i16 = mybir.dt.int16
    AF = mybir.ActivationFunctionType
    ALU = mybir.AluOpType

    nc = bacc.Bacc(None, target_bir_lowering=False)

    # ---- I/O ----
    x_aug_d = nc.declare_dram_parameter("x_aug", [DAUG, TB], bf16, isOutput=False)
    x_c_d = nc.declare_dram_parameter("x_c", [D, TC], bf16, isOutput=False)
    wqkv_d = nc.declare_dram_parameter("wqkv_aug", [DAUG, 6 * P], bf16, isOutput=False)
    wproj_d = nc.declare_dram_parameter("wproj_c", [2 * P, D], bf16, isOutput=False)
    w1_d = nc.declare_dram_parameter("w1_aug", [D, FF], bf16, isOutput=False)
    w2t_d = nc.declare_dram_parameter("w2t", [FF, D], bf16, isOutput=False)
    aux_d = nc.declare_dram_parameter("aux", [P, 64], f32, isOutput=False)
    # aux columns: 0:8 -> b_proj as [128,8], 8:40 -> b1 as [128,32],
    # 40:48 -> b2 as [128,8], 48:54 -> C1 (qkv bias-fold) as [128,6]
    out_d = nc.declare_dram_parameter("out_c", [D, TC], f32, isOutput=True)

    groups = [[0, 1, 2, 3], [4, 5, 6, 7]]

    with tile.TileContext(nc) as tc, ExitStack() as ctx:
        const = ctx.enter_context(tc.tile_pool(name="const", bufs=1))
        dram = ctx.enter_context(tc.tile_pool(name="dram", bufs=1, space="DRAM"))

        ident = const.tile([P, P], bf16)
        make_identity(nc, ident)
        ones128 = const.tile([P, P], bf16)
        nc.any.memset(ones128, 1.0)
        eps_col = const.tile([P, 1], f32)
        nc.any.memset(eps_col, EPS)

        wqkv_sb = const.tile([P, KA, 6 * P], bf16)
        nc.sync.dma_start(wqkv_sb, wqkv_d.rearrange("(k p) e -> p k e", p=P))
        wproj_sb = const.tile([P, 2, D], bf16)
        nc.sync.dma_start(wproj_sb, wproj_d.rearrange("(k p) d -> p k d", p=P))
        aux_sb = const.tile([P, 64], f32)
        nc.sync.dma_start(aux_sb, aux_d[:])

        # long-lived activation tensors
        x1grp = ctx.enter_context(tc.tile_pool(name="x1grp", bufs=1))
        x1aug = x1grp.tile([P, KT, TC], bf16)
        work = ctx.enter_context(tc.tile_pool(name="work", bufs=1))

        psA = ctx.enter_context(tc.tile_pool(name="psA", bufs=2, space="PSUM"))

        # residual input, prefetched during phase A / attention
        resid = ctx.enter_context(tc.tile_pool(name="resid", bufs=1))
        xc = resid.tile([P, KT, TC], bf16, tag="xc")
        nc.sync.dma_start(xc, x_c_d.rearrange("(k p) t -> p k t", p=P))
        xb = resid.tile([P, KT, TC], bf16, tag="xb")

        # w1 weights, prefetched during attention
        w1_pool = ctx.enter_context(tc.tile_pool(name="w1pool", bufs=1))

        # proj partials per query chunk, wide-row layout for the collective:
        # row r*128 + p, col m*128 + t  <->  feature m*128+p, rank-r slab
        # token t (2KB rows so the ReduceScatter moves efficient lines)
        partial_d = [
            dram.tile([4 * P, KT * SLAB], bf16, tag=f"pp{qc}", name=f"pp{qc}")
            for qc in range(NQC)
        ]
        x1p_d = [
            dram.tile([P, KT * SLAB], bf16, tag=f"xp{qc}", name=f"xp{qc}")
            for qc in range(NQC)
        ]

        x_aug_r = x_aug_d.rearrange("(k p) t -> p k t", p=P)
        w1_noaug_r = w1_d.rearrange("(k p) f -> p k f", p=P)

        w1q = []
        with tc.tile_pool(name="qkvTp", bufs=1) as qkvT_pool, \
             tc.tile_pool(name="attnTp", bufs=1) as attnT_pool, \
             tc.tile_pool(name="attg", bufs=1) as attg, \
             tc.tile_pool(name="etp", bufs=7) as etp, \
             tc.tile_pool(name="poutp", bufs=3) as poutp, \
             tc.tile_pool(name="lgp", bufs=3, space="PSUM") as lgp, \
             tc.tile_pool(name="avqp", bufs=2, space="PSUM") as avqp, \
             tc.tile_pool(name="rbpp", bufs=1, space="PSUM") as rbpp:
            qkvT = [qkvT_pool.tile([P, 2, TB], bf16, name=f"qkvT{pt}") for pt in (0, 1)]
            attnT = [attnT_pool.tile([P, TB], bf16, name=f"attnT{pt}") for pt in (0, 1)]
            # vext: per key tile: [h0 | 1 | h1 | 1 | h2 | 1 | h3 | 1]
            vext = attg.tile([P, NKT, 4 * 65], bf16)

            # ============ phase A: LN1 stats + qkv + vext, per token chunk ===
            with tc.tile_pool(name="xaug", bufs=2) as xaug_pool, \
                 tc.tile_pool(name="workA", bufs=2) as workA, \
                 nc.named_scope("ln1_qkv"):
                for hp in range(4):
                    nc.any.memset(vext[:, :, hp * 65 + 64 : hp * 65 + 65], 1.0)
                for tch in range(NQC):
                    tsl = slice(tch * 512, (tch + 1) * 512)
                    xa = xaug_pool.tile([P, KA, 512], bf16, tag="xa")
                    nc.sync.dma_start(xa, x_aug_r[:, :, tsl])
                    pmu = psA.tile([P, 512], f32, tag="a", name="pmu")
                    psq = psA.tile([P, 512], f32, tag="a", name="psq")
                    for kt in range(KT):
                        xsq = workA.tile([P, 512], bf16, tag="xsq")
                        nc.vector.tensor_tensor(
                            xsq, xa[:, kt, :], xa[:, kt, :], ALU.mult
                        )
                        nc.tensor.matmul(
                            pmu, ones128, xa[:, kt, :],
                            start=(kt == 0), stop=(kt == KT - 1),
                        )
                        nc.tensor.matmul(
                            psq, ones128, xsq,
                            start=(kt == 0), stop=(kt == KT - 1),
                        )
                    m1 = workA.tile([P, 512], f32, tag="m1")
                    nc.vector.tensor_scalar_mul(m1, pmu, 1.0 / D)
                    # augmented row: -mu (bf16), partition 0 of k-tile 8
                    nc.vector.tensor_scalar_mul(xa[0:1, KT, :], m1[0:1, :], -1.0)
                    v1 = workA.tile([P, 512], f32, tag="v1")
                    nc.vector.tensor_scalar_mul(v1, psq, 1.0 / D)
                    m2 = workA.tile([P, 512], f32, tag="m2")
                    nc.vector.tensor_tensor(m2, m1, m1, ALU.mult)
                    nc.vector.tensor_tensor(v1, v1, m2, ALU.subtract)
                    sd = workA.tile([P, 512], f32, tag="sd")
                    nc.scalar.activation(sd, v1, AF.Sqrt, bias=eps_col)
                    r1b = xaug_pool.tile([P, 512], f32, tag="r1b")
                    nc.vector.reciprocal_approx_fast(r1b, sd)

                    vtmp = [None, None]
                    for pt in range(2):
                        for m in range(3):
                            msl = slice(pt * 3 * P + m * P, pt * 3 * P + (m + 1) * P)
                            ps = lgp.tile([P, 512], f32, tag="lg", name="qkvps")
                            for kt in range(KA):
                                nc.tensor.matmul(
                                    ps, wqkv_sb[:, kt, msl], xa[:, kt, :],
                                    start=(kt == 0), stop=(kt == KA - 1),
                                )
                            if m < 2:
                                dst = qkvT[pt][:, m, tsl]
                            else:
                                vtmp[pt] = etp.tile(
                                    [P, 512], bf16, tag="et", name=f"vtmp{pt}"
                                )
                                dst = vtmp[pt]
                            nc.vector.tensor_tensor(dst, ps, r1b, ALU.mult)
                            if has_c1:
                                nc.vector.tensor_scalar(
                                    dst, dst,
                                    aux_sb[:, 48 + pt * 3 + m : 49 + pt * 3 + m],
                                    None, ALU.add,
                                )
                    # vext for this chunk's 4 key tiles
                    with nc.named_scope("vext"):
                        for k4 in range(4):
                            kt = tch * 4 + k4
                            for pt in range(2):
                                pt_t = psA.tile([P, 512], bf16, tag="a", name="ptt")[
                                    :, 0:P
                                ]
                                nc.tensor.transpose(
                                    pt_t, vtmp[pt][:, k4 * P : (k4 + 1) * P], ident
                                )
                                c0 = pt * 130
                                nc.vector.tensor_copy(
                                    vext[:, kt, c0 : c0 + 64], pt_t[:, 0:64]
                                )
                                nc.vector.tensor_copy(
                                    vext[:, kt, c0 + 65 : c0 + 129], pt_t[:, 64:128]
                                )

            # prefetch MLP-up weights during attention
            FQ = FF // 4
            for q in range(4):
                w1qt = w1_pool.tile([P, KT, FQ], bf16, tag=f"w1_{q}", name=f"w1q{q}")
                nc.sync.dma_start(w1qt, w1_noaug_r[:, :, q * FQ : (q + 1) * FQ])
                w1q.append(w1qt)

            # ============ phase B: attention ================================
            with nc.named_scope("attn"):
                from collections import deque

                epi_q = deque()
                epi_b = deque()
                proj_q = deque()

                def _epilogue_a(st):
                    pt, hp, qc, avq = st
                    rs_sb = attg.tile([1, 512], f32, tag="rsb", name="rs_sb", bufs=2)
                    nc.scalar.activation(rs_sb, avq[64:65, :], AF.Copy)
                    rc_f = attg.tile([1, 512], f32, tag="rcf", name="rcf", bufs=2)
                    nc.vector.reciprocal_approx_fast(rc_f, rs_sb)
                    rc_b = attg.tile([1, 512], bf16, tag="rcb", name="rcb", bufs=2)
                    nc.scalar.activation(rc_b, rc_f, AF.Copy)
                    return (pt, hp, qc, avq, rc_b)

                def _epilogue_b(st):
                    pt, hp, qc, avq, rc_b = st
                    q0 = qc * 512
                    rbp = rbpp.tile([P, 512], f32, tag="rb", name="rbp")[0:64, :]
                    nc.tensor.matmul(
                        rbp, ones128[0:1, 0:64], rc_b, start=True, stop=True
                    )
                    rbs = attg.tile([64, 512], bf16, tag="rbs", name="rbs", bufs=2)
                    nc.scalar.activation(rbs, rbp, AF.Copy)
                    nc.vector.tensor_tensor(
                        attnT[pt][hp * HD : (hp + 1) * HD, q0 : q0 + 512],
                        avq[0:64, :], rbs, ALU.mult,
                    )

                def _emit_proj():
                    qc, m = proj_q.popleft()
                    tsl = slice(qc * 512, (qc + 1) * 512)
                    ps = psA.tile([P, 512], f32, tag="a", name="projps")
                    for kt2 in range(2):
                        nc.tensor.matmul(
                            ps, wproj_sb[:, kt2, m * P : (m + 1) * P],
                            attnT[kt2][:, tsl], start=(kt2 == 0), stop=(kt2 == 1),
                        )
                    pb = poutp.tile([P, 512], bf16, tag="pout", name="pb")
                    nc.scalar.activation(pb, ps, AF.Copy)
                    # scatter: rank r's slab columns -> rows r*128.., col m*128..
                    for r in range(4):
                        nc.sync.dma_start(
                            partial_d[qc][
                                r * P : (r + 1) * P, m * SLAB : (m + 1) * SLAB
                            ],
                            pb[:, r * SLAB : (r + 1) * SLAB],
                        )
                    if m == KT - 1:
                        with nc.named_scope("reducescatter"):
                            nc.gpsimd.collective_compute(
                                "ReduceScatter",
                                mybir.AluOpType.add,
                                replica_groups=groups,
                                ins=[partial_d[qc][:]],
                                outs=[x1p_d[qc][:]],
                            )

                for qc in range(NQC):
                    for u in range(4):
                        pt, hp = u // 2, u % 2
                        hsl = slice(hp * HD, (hp + 1) * HD)
                        q0 = qc * 512
                        avq = avqp.tile([P, 512], f32, tag="avq",
                                        name=f"avq{qc}{u}")[0:65, :]
                        for kt in range(NKT):
                            ksl = slice(kt * P, (kt + 1) * P)
                            lg = lgp.tile([P, 512], f32, tag="lg", name="lg")
                            nc.tensor.matmul(
                                lg, qkvT[pt][hsl, 1, ksl],
                                qkvT[pt][hsl, 0, q0 : q0 + 512],
                                start=True, stop=True,
                            )
                            et = etp.tile([P, 512], bf16, tag="et")
                            if kt in SC_KT:
                                nc.scalar.activation(
                                    et, lg, AF.Exp, scale=1.0 / np.sqrt(HD)
                                )
                            else:
                                nc.vector.tensor_scalar(
                                    et.bitcast(i16), lg, EXP_A, EXP_B,
                                    ALU.mult, ALU.add,
                                )
                            vcol = slice(pt * 130 + hp * 65, pt * 130 + hp * 65 + 65)
                            nc.tensor.matmul(
                                avq, vext[:, kt, vcol], et,
                                start=(kt == 0), stop=(kt == NKT - 1),
                            )
                            if kt == 1 and epi_q:
                                epi_b.append(_epilogue_a(epi_q.popleft()))
                            if kt == 6 and epi_b:
                                _epilogue_b(epi_b.popleft())
                            if u > 0 and kt in (3, 7, 11) and proj_q:
                                _emit_proj()
                        epi_q.append((pt, hp, qc, avq))
                    proj_q.extend((qc, m) for m in range(8))

                while epi_q:
                    epi_b.append(_epilogue_a(epi_q.popleft()))
                while epi_b:
                    _epilogue_b(epi_b.popleft())
                with nc.named_scope("proj"):
                    while proj_q:
                        _emit_proj()

        # ============ phase C + D: residual/LN2 per slab + 2-pass MLP =====
        w_stack = ExitStack()
        w2_pool = w_stack.enter_context(tc.tile_pool(name="w2pool", bufs=1))
        psB = w_stack.enter_context(tc.tile_pool(name="psB", bufs=1, space="PSUM"))
        pcs = w_stack.enter_context(tc.tile_pool(name="pcs", bufs=2, space="PSUM"))
        H2S = 24  # h2 ring slots (down trails up by 16 f-tiles)
        h2T = w_stack.enter_context(tc.tile_pool(name="h2", bufs=1)).tile(
            [P, H2S, TC], bf16
        )
        x1pp = w_stack.enter_context(tc.tile_pool(name="x1pp", bufs=2))

        def _phase_c_slab(qc):
            csl = slice(qc * SLAB, (qc + 1) * SLAB)
            x1p = x1pp.tile([P, KT, SLAB], bf16, tag="x1p", name=f"x1p{qc}")
            nc.sync.dma_start(x1p, x1p_d[qc].rearrange("p (k t) -> p k t", k=KT))
            # pmu/psq share one PSUM bank: a single accumulation group with
            # start only on the very first matmul (start zeroes the whole
            # 2KB bank) and stop on the last
            stat = pcs.tile([P, 512], f32, tag="s", name=f"stat{qc}")
            pmu = stat[:, 0:SLAB]
            psq = stat[:, SLAB : 2 * SLAB]
            for kt in range(KT):
                nc.vector.tensor_tensor(
                    xb[:, kt, csl], xc[:, kt, csl], x1p[:, kt, :], ALU.add
                )
                if has_bproj:
                    nc.vector.tensor_scalar(
                        xb[:, kt, csl], xb[:, kt, csl],
                        aux_sb[:, kt : kt + 1], None, ALU.add,
                    )
                xsq = work.tile([P, SLAB], bf16, tag="xsq")
                nc.vector.tensor_tensor(
                    xsq, xb[:, kt, csl], xb[:, kt, csl], ALU.mult
                )
                nc.tensor.matmul(
                    pmu, ones128, xb[:, kt, csl],
                    start=(kt == 0), stop=False, skip_group_check=True,
                )
                nc.tensor.matmul(
                    psq, ones128, xsq, start=False,
                    stop=(kt == KT - 1), skip_group_check=True,
                )
            m1 = work.tile([P, SLAB], f32, tag="m1")
            nc.vector.tensor_scalar_mul(m1, pmu, 1.0 / D)
            v1 = work.tile([P, SLAB], f32, tag="v1")
            nc.vector.tensor_scalar_mul(v1, psq, 1.0 / D)
            m2 = work.tile([P, SLAB], f32, tag="m2")
            nc.vector.tensor_tensor(m2, m1, m1, ALU.mult)
            nc.vector.tensor_tensor(v1, v1, m2, ALU.subtract)
            sd = work.tile([P, SLAB], f32, tag="sd")
            nc.scalar.activation(sd, v1, AF.Sqrt, bias=eps_col)
            r2b = work.tile([P, SLAB], f32, tag="xsq")
            nc.vector.reciprocal_approx_fast(r2b, sd)
            m1b = work.tile([P, SLAB], bf16, tag="m1b")
            nc.scalar.activation(m1b, m1, AF.Copy)
            r2s = work.tile([P, SLAB], bf16, tag="r2s")
            nc.scalar.activation(r2s, r2b, AF.Copy)
            # x1aug = (x1 - mu)/sigma in bf16
            for kt in range(KT):
                nc.vector.tensor_tensor(
                    x1aug[:, kt, csl], xb[:, kt, csl], m1b, ALU.subtract
                )
                nc.vector.tensor_tensor(
                    x1aug[:, kt, csl], x1aug[:, kt, csl], r2s, ALU.mult
                )

        NF = FF // P  # 32 f-tiles
        NQ = NF // 4  # 8 f-tiles per weight quarter
        w2r = w2t_d.rearrange("(k p) d -> p k d", p=P)
        w2q = [None] * 4

        def _w2s(kt, m):
            return w2q[kt // NQ][:, kt % NQ, m * P : (m + 1) * P]

        assert not has_c2, "nonzero ln2_b not supported"
        HTC = TC // 2  # 256 tokens per MLP pass
        with nc.named_scope("x1_ln2"):
            for qc in range(3):
                _phase_c_slab(qc)

        def _accs(sfx):
            acc4 = [
                psB.tile([P, 2 * HTC], f32, tag=f"acc{g}", name=f"m2{sfx}{g}")
                for g in range(4)
            ]
            return [
                acc4[m // 2][:, (m % 2) * HTC : (m % 2 + 1) * HTC]
                for m in range(KT)
            ]

        accs = _accs("p")
        with nc.named_scope("mlp"):
            for p_i in range(2):
                t0 = p_i * HTC
                tsl = slice(t0, t0 + HTC)
                if p_i == 1:
                    accs = _accs("q")
                for j in range(NF):
                    if p_i == 0 and j >= NQ and j % NQ == 0:
                        q = j // NQ - 1
                        w2q[q] = w2_pool.tile(
                            [P, NQ, D], bf16, tag=f"w2_{q}", name=f"w2q{q}"
                        )
                        nc.sync.dma_start(w2q[q], w2r[:, q * NQ : (q + 1) * NQ, :])
                    if p_i == 0 and j == 20:
                        # slab 3's residual+LN2, gated on the last collective
                        with nc.named_scope("x1_ln2_s3"):
                            _phase_c_slab(3)
                    w1h = w1q[j // NQ]
                    msl = slice((j % NQ) * P, (j % NQ + 1) * P)
                    ps = psA.tile([P, HTC], f32, tag="a", name="m1ps")
                    for kt in range(KT):
                        nc.tensor.matmul(
                            ps, w1h[:, kt, msl], x1aug[:, kt, tsl],
                            start=(kt == 0), stop=(kt == KT - 1),
                        )
                    bias_arg = aux_sb[:, 8 + j : 9 + j] if has_b1 else 0.0
                    nc.scalar.activation(
                        h2T[:, j % H2S, tsl], ps, AF.Relu, bias=bias_arg
                    )
                    if j >= 2 * NQ:
                        kt2 = j - 2 * NQ
                        for m in range(KT):
                            nc.tensor.matmul(
                                accs[m], _w2s(kt2, m), h2T[:, kt2 % H2S, tsl],
                                start=(kt2 == 0 and m % 2 == 0), stop=False,
                                skip_group_check=True,
                            )
                if p_i == 0:
                    w2q[3] = w2_pool.tile([P, NQ, D], bf16, tag="w2_3", name="w2q3")
                    nc.sync.dma_start(w2q[3], w2r[:, 3 * NQ :, :])
                for kt2 in range(NF - 2 * NQ, NF):
                    for m in range(KT):
                        nc.tensor.matmul(
                            accs[m], _w2s(kt2, m), h2T[:, kt2 % H2S, tsl],
                            start=False,
                            stop=(kt2 == NF - 1 and m % 2 == 1),
                            skip_group_check=True,
                        )
                for m in range(KT):
                    ob = work.tile([P, HTC], f32, tag="ob", bufs=2)
                    nc.vector.tensor_tensor(ob, accs[m], xb[:, m, tsl], ALU.add)
                    if has_b2:
                        nc.vector.tensor_scalar(
                            ob, ob, aux_sb[:, 40 + m : 41 + m], None, ALU.add
                        )
                    nc.sync.dma_start(out_d[m * P : (m + 1) * P, tsl], ob)
        w_stack.close()

    nc.compile()
    return nc


def _slab_cols(c):
    """Column indices into xT [D, T] owned by core c, in kernel order."""
    bc, r = c // 4, c % 4
    cols = []
    for qc in range(NQC):
        base = bc * TB + qc * 512 + r * SLAB
        cols.append(np.arange(base, base + SLAB))
    return np.concatenate(cols)


def _prep_inputs(inputs):
    x = np.asarray(inputs["x"], np.float32)
    w_qkv = np.asarray(inputs["w_qkv"], np.float32)
    w_proj = np.asarray(inputs["w_proj"], np.float32)
    b_proj = np.asarray(inputs["b_proj"], np.float32)
    w1 = np.asarray(inputs["w1"], np.float32)
    b1 = np.asarray(inputs["b1"], np.float32)
    w2 = np.asarray(inputs["w2"], np.float32)
    b2 = np.asarray(inputs["b2"], np.float32)
    ln1_g = np.asarray(inputs["ln1_g"], np.float32)
    ln1_b = np.asarray(inputs["ln1_b"], np.float32)
    ln2_g = np.asarray(inputs["ln2_g"], np.float32)
    ln2_b = np.asarray(inputs["ln2_b"], np.float32)

    has_c1 = bool(np.any(ln1_b != 0))
    has_bproj = bool(np.any(b_proj != 0))
    has_c2 = bool(np.any(ln2_b != 0))
    has_b1 = bool(np.any(b1 != 0))
    has_b2 = bool(np.any(b2 != 0))
    flags = (has_c1, has_bproj, has_c2, has_b1, has_b2)

    xT = np.ascontiguousarray(x.reshape(T, D).T)  # [D, T] f32

    wg = w_qkv * ln1_g[None, :]  # [3D, D]
    Se = wg.sum(axis=1)  # [3D]
    Ce = w_qkv @ ln1_b  # [3D]
    w1g = w1 * ln2_g[None, :]  # [FF, D]
    C2 = w1 @ ln2_b
    if np.any(C2 != 0):
        raise NotImplementedError("nonzero ln2_b not supported")

    w1_aug = np.ascontiguousarray(w1g.T).astype(BF16)
    w2t = np.ascontiguousarray(w2.T).astype(BF16)  # [FF, D]

    in_maps = []
    for c in range(NCORES):
        bc, hg = c // 4, c % 4
        # batch-sliced augmented x
        x_aug = np.zeros((DAUG, TB), BF16)
        x_aug[:D] = xT[:, bc * TB : (bc + 1) * TB].astype(BF16)

        # qkv weights for 4 heads: two partition-tiles of head pairs
        wqkv_aug = np.zeros((DAUG, 6 * P), BF16)
        cstack = np.zeros((P, 6), np.float32)
        for pt in range(2):
            r0 = (4 * hg + 2 * pt) * HD  # 128 contiguous rows (2 heads)
            for m in range(3):
                rows = slice(m * D + r0, m * D + r0 + 2 * HD)
                csl = slice(pt * 3 * P + m * P, pt * 3 * P + (m + 1) * P)
                wqkv_aug[:D, csl] = wg[rows].T.astype(BF16)
                wqkv_aug[D, csl] = Se[rows].astype(BF16)
                cstack[:, pt * 3 + m] = Ce[rows]

        # proj rows for this core's 256 head dims
        wproj_c = np.ascontiguousarray(
            w_proj[:, 4 * hg * HD : (4 * hg + 4) * HD].T
        ).astype(BF16)  # [256, D]

        aux = np.zeros((P, 64), np.float32)
        aux[:, 0:8] = b_proj.reshape(KT, P).T
        aux[:, 8:40] = b1.reshape(FF // P, P).T
        aux[:, 40:48] = b2.reshape(KT, P).T
        aux[:, 48:54] = cstack

        in_maps.append(
            {
                "x_aug": x_aug,
                "x_c": np.ascontiguousarray(xT[:, _slab_cols(c)]).astype(BF16),
                "wqkv_aug": wqkv_aug,
                "wproj_c": wproj_c,
                "w1_aug": w1_aug,
                "w2t": w2t,
                "aux": aux,
            }
        )
    return flags, in_maps


def _run(inputs, trace=False, trace_kwargs=None):
    from concourse.bass_utils import run_bass_kernel_spmd

    flags, in_maps = _prep_inputs(inputs)
    if flags not in _CACHE:
        _CACHE[flags] = _build_program(*flags)
    nc = _CACHE[flags]
    res = run_bass_kernel_spmd(
        nc, in_maps, list(range(NCORES)), trace=trace,
        **(trace_kwargs or {}),
    )
    outT = np.empty((D, T), np.float32)
    for c in range(NCORES):
        outT[:, _slab_cols(c)] = res.results[c]["out_c"]
    out = np.ascontiguousarray(outT.T).reshape(B, S, D)
    return out, res


def kernel(**inputs):
    out, _ = _run(inputs, trace=False)
    return out


# revision 31
# speedup vs baseline: 1.1341x; 1.0996x over previous
"""Trainium2 Bass kernel for a pre-LN transformer block (B=2, S=2048, D=1024,
H=16, d_ff=4096), 8-way (batch, head-group) tensor-parallel:

- core c handles batch c//4 and heads 4*(c%4)..4*(c%4)+3: LN1+qkv run over the
  core's 2048 batch tokens only, attention over 4 heads
- softmax exp is split across engines: even key-tiles use the Activation
  engine's exact Exp, odd key-tiles use a Schraudolph-style int16 exponent
  construction on the DVE (bitcast to bf16)
- attention-proj partials are ReduceScattered per query-chunk (4 collectives),
  each fired as soon as that chunk's proj partials are done, so 3 of 4 overlap
  the remaining attention compute; each core owns four interleaved 128-token
  slabs (slab qc = tokens qc*512 + rank*128 ..+128) so the residual+LN2+MLP
  pipeline starts at attention end, with the MLP split into two 256-token
  passes (the second gated only on the last collective)
- token-sharded MLP with the full d_ff on each core (no second collective)

Activations live feature-major [feature, token].  LayerNorm is folded into the
matmuls via an augmented contraction row (-mu) and column (row-sums of the
g-scaled weights); the 1/sigma factor is applied on PSUM eviction.  Softmax is
computed unnormalized with a ones-column appended to V producing row sums, and
1/sum is applied on the attention-output eviction.
"""

import sys

for _p in ("/opt/trn_rl_repo",):
    if _p not in sys.path:
        sys.path.insert(0, _p)

import numpy as np
import ml_dtypes

B, S, D = 2, 2048, 1024
H, HD = 16, 64
FF = 4 * D
T = B * S  # 4096 tokens
NCORES = 8
TC = T // NCORES  # 512 tokens per core (MLP/out shard)
TB = S  # 2048 tokens per batch (per-core attention range)
P = 128
KT = D // P  # 8 k-tiles over D
KA = 9  # augmented k-tiles
DAUG = D + P  # 1152
EPS = 1e-5
NKT = TB // P  # 16 key tiles per batch
NQC = TB // 512  # 4 q-chunks of 512
SLAB = TC // NQC  # 128 tokens per owned slab
BF16 = ml_dtypes.bfloat16

# Schraudolph exp: bf16 bits ~= round(x*log2(e)*128 + (127*128 - 7.63))
LOG2E = float(np.log2(np.e))
EXP_A = 128.0 * LOG2E / np.sqrt(HD)  # logit scale 1/sqrt(HD) folded in
EXP_B = 127.0 * 128.0 - 7.63
# key tiles using exact Exp on the Activation engine (rest: Schraudolph on DVE)
SC_KT = frozenset({0, 2, 4, 6, 8, 10, 12, 14})

_CACHE = {}


def _build_program(has_c1, has_bproj, has_c2, has_b1, has_b2):
    import concourse.mybir as mybir
    import concourse.tile as tile
    from concourse import bacc
    from concourse.masks import make_identity
    from contextlib import ExitStack

    f32 = mybir.dt.float32
    bf16 = mybir.dt.bfloat16
    i16 = mybir.dt.int16
    AF = mybir.ActivationFunctionType
    ALU = mybir.AluOpType

    nc = bacc.Bacc(None, target_bir_lowering=False)

    # ---- I/O ----
    x_aug_d = nc.declare_dram_parameter("x_aug", [DAUG, TB], bf16, isOutput=False)
    x_c_d = nc.declare_dram_parameter("x_c", [D, TC], bf16, isOutput=False)
    wqkv_d = nc.declare_dram_parameter("wqkv_aug", [DAUG, 6 * P], bf16, isOutput=False)
    wproj_d = nc.declare_dram_parameter("wproj_c", [2 * P, D], bf16, isOutput=False)
    w1_d = nc.declare_dram_parameter("w1_aug", [D, FF], bf16, isOutput=False)
    w2t_d = nc.declare_dram_parameter("w2t", [FF, D], bf16, isOutput=False)
    aux_d = nc.declare_dram_parameter("aux", [P, 64], f32, isOutput=False)
    # aux columns: 0:8 -> b_proj as [128,8], 8:40 -> b1 as [128,32],
    # 40:48 -> b2 as [128,8], 48:54 -> C1 (qkv bias-fold) as [128,6]
    out_d = nc.declare_dram_parameter("out_c", [D, TC], f32, isOutput=True)

    groups = [[0, 1, 2, 3], [4, 5, 6, 7]]

    with tile.TileContext(nc) as tc, ExitStack() as ctx:
        const = ctx.enter_context(tc.tile_pool(name="const", bufs=1))
        dram = ctx.enter_context(tc.tile_pool(name="dram", bufs=1, space="DRAM"))

        ident = const.tile([P, P], bf16)
        make_identity(nc, ident)
        ones128 = const.tile([P, P], bf16)
        nc.any.memset(ones128, 1.0)
        eps_col = const.tile([P, 1], f32)
        nc.any.memset(eps_col, EPS)

        wqkv_sb = const.tile([P, KA, 6 * P], bf16)
        nc.sync.dma_start(wqkv_sb, wqkv_d.rearrange("(k p) e -> p k e", p=P))
        wproj_sb = const.tile([P, 2, D], bf16)
        nc.sync.dma_start(wproj_sb, wproj_d.rearrange("(k p) d -> p k d", p=P))
        aux_sb = const.tile([P, 64], f32)
        nc.sync.dma_start(aux_sb, aux_d[:])

        # long-lived activation tensors
        x1grp = ctx.enter_context(tc.tile_pool(name="x1grp", bufs=1))
        x1aug = x1grp.tile([P, KT, TC], bf16)
        work = ctx.enter_context(tc.tile_pool(name="work", bufs=1))

        psA = ctx.enter_context(tc.tile_pool(name="psA", bufs=2, space="PSUM"))

        # residual input, prefetched during phase A / attention
        resid = ctx.enter_context(tc.tile_pool(name="resid", bufs=1))
        xc = resid.tile([P, KT, TC], bf16, tag="xc")
        nc.sync.dma_start(xc, x_c_d.rearrange("(k p) t -> p k t", p=P))
        xb = resid.tile([P, KT, TC], bf16, tag="xb")

        # w1 weights, prefetched during attention
        w1_pool = ctx.enter_context(tc.tile_pool(name="w1pool", bufs=1))

        # proj partials per query chunk, wide-row layout for the collective:
        # row r*128 + p, col m*128 + t  <->  feature m*128+p, rank-r slab
        # token t (2KB rows so the ReduceScatter moves efficient lines)
        partial_d = [
            dram.tile([4 * P, KT * SLAB], bf16, tag=f"pp{qc}", name=f"pp{qc}")
            for qc in range(NQC)
        ]
        x1p_d = [
            dram.tile([P, KT * SLAB], bf16, tag=f"xp{qc}", name=f"xp{qc}")
            for qc in range(NQC)
        ]

        x_aug_r = x_aug_d.rearrange("(k p) t -> p k t", p=P)
        w1_noaug_r = w1_d.rearrange("(k p) f -> p k f", p=P)

        w1q = []
        with tc.tile_pool(name="qkvTp", bufs=1) as qkvT_pool, \
             tc.tile_pool(name="attnTp", bufs=1) as attnT_pool, \
             tc.tile_pool(name="attg", bufs=1) as attg, \
             tc.tile_pool(name="etp", bufs=7) as etp, \
             tc.tile_pool(name="poutp", bufs=3) as poutp, \
             tc.tile_pool(name="lgp", bufs=3, space="PSUM") as lgp, \
             tc.tile_pool(name="avqp", bufs=3, space="PSUM") as avqp:
            qkvT = [qkvT_pool.tile([P, 2, TB], bf16, name=f"qkvT{pt}") for pt in (0, 1)]
            attnT = [attnT_pool.tile([P, TB], bf16, name=f"attnT{pt}") for pt in (0, 1)]
            # vext: per key tile: [h0 | 1 | h1 | 1 | h2 | 1 | h3 | 1]
            vext = attg.tile([P, NKT, 4 * 65], bf16)

            # ============ phase A: LN1 stats + qkv + vext, per token chunk ===
            with tc.tile_pool(name="xaug", bufs=2) as xaug_pool, \
                 tc.tile_pool(name="workA", bufs=2) as workA, \
                 nc.named_scope("ln1_qkv"):
                for hp in range(4):
                    nc.any.memset(vext[:, :, hp * 65 + 64 : hp * 65 + 65], 1.0)
                for tch in range(NQC):
                    tsl = slice(tch * 512, (tch + 1) * 512)
                    xa = xaug_pool.tile([P, KA, 512], bf16, tag="xa")
                    nc.sync.dma_start(xa, x_aug_r[:, :, tsl])
                    pmu = psA.tile([P, 512], f32, tag="a", name="pmu")
                    psq = psA.tile([P, 512], f32, tag="a", name="psq")
                    for kt in range(KT):
                        xsq = workA.tile([P, 512], bf16, tag="xsq")
                        nc.vector.tensor_tensor(
                            xsq, xa[:, kt, :], xa[:, kt, :], ALU.mult
                        )
                        nc.tensor.matmul(
                            pmu, ones128, xa[:, kt, :],
                            start=(kt == 0), stop=(kt == KT - 1),
                        )
                        nc.tensor.matmul(
                            psq, ones128, xsq,
                            start=(kt == 0), stop=(kt == KT - 1),
                        )
                    m1 = workA.tile([P, 512], f32, tag="m1")
                    nc.vector.tensor_scalar_mul(m1, pmu, 1.0 / D)
                    # augmented row: -mu (bf16), partition 0 of k-tile 8
                    nc.vector.tensor_scalar_mul(xa[0:1, KT, :], m1[0:1, :], -1.0)
                    v1 = workA.tile([P, 512], f32, tag="v1")
                    nc.vector.tensor_scalar_mul(v1, psq, 1.0 / D)
                    m2 = workA.tile([P, 512], f32, tag="m2")
                    nc.vector.tensor_tensor(m2, m1, m1, ALU.mult)
                    nc.vector.tensor_tensor(v1, v1, m2, ALU.subtract)
                    sd = workA.tile([P, 512], f32, tag="sd")
                    nc.scalar.activation(sd, v1, AF.Sqrt, bias=eps_col)
                    r1b = xaug_pool.tile([P, 512], f32, tag="r1b")
                    nc.vector.reciprocal_approx_fast(r1b, sd)

                    vtmp = [None, None]
                    for pt in range(2):
                        for m in range(3):
                            msl = slice(pt * 3 * P + m * P, pt * 3 * P + (m + 1) * P)
                            ps = lgp.tile([P, 512], f32, tag="lg", name="qkvps")
                            for kt in range(KA):
                                nc.tensor.matmul(
                                    ps, wqkv_sb[:, kt, msl], xa[:, kt, :],
                                    start=(kt == 0), stop=(kt == KA - 1),
                                )
                            if m < 2:
                                dst = qkvT[pt][:, m, tsl]
                            else:
                                vtmp[pt] = etp.tile(
                                    [P, 512], bf16, tag="et", name=f"vtmp{pt}"
                                )
                                dst = vtmp[pt]
                            nc.vector.tensor_tensor(dst, ps, r1b, ALU.mult)
                            if has_c1:
                                nc.vector.tensor_scalar(
                                    dst, dst,
                                    aux_sb[:, 48 + pt * 3 + m : 49 + pt * 3 + m],
                                    None, ALU.add,
                                )
                    # vext for this chunk's 4 key tiles
                    with nc.named_scope("vext"):
                        for k4 in range(4):
                            kt = tch * 4 + k4
                            for pt in range(2):
                                pt_t = psA.tile([P, 512], bf16, tag="a", name="ptt")[
                                    :, 0:P
                                ]
                                nc.tensor.transpose(
                                    pt_t, vtmp[pt][:, k4 * P : (k4 + 1) * P], ident
                                )
                                c0 = pt * 130
                                nc.vector.tensor_copy(
                                    vext[:, kt, c0 : c0 + 64], pt_t[:, 0:64]
                                )
                                nc.vector.tensor_copy(
                                    vext[:, kt, c0 + 65 : c0 + 129], pt_t[:, 64:128]
                                )

            # prefetch MLP-up weights during attention
            FQ = FF // 4
            for q in range(4):
                w1qt = w1_pool.tile([P, KT, FQ], bf16, tag=f"w1_{q}", name=f"w1q{q}")
                nc.sync.dma_start(w1qt, w1_noaug_r[:, :, q * FQ : (q + 1) * FQ])
                w1q.append(w1qt)

            # ---- residual + LN2 pipeline, staged so it can interleave with
            # attention.  SBUF-only elementwise goes to the (idle) GpSimd
            # engine; PSUM reads stay on DVE/Scalar.
            slab_state = {}

            def _slab_s1(s, pool):
                csl = slice(s * SLAB, (s + 1) * SLAB)
                x1p = pool.tile([P, KT, SLAB], bf16, tag="x1p", name=f"x1p{s}")
                nc.sync.dma_start(
                    x1p, x1p_d[s].rearrange("p (k t) -> p k t", k=KT)
                )
                xsqs = pool.tile([P, KT, SLAB], bf16, tag="xsqs", name=f"xsqs{s}")
                for kt in range(KT):
                    nc.gpsimd.tensor_tensor(
                        xb[:, kt, csl], xc[:, kt, csl], x1p[:, kt, :], ALU.add
                    )
                    if has_bproj:
                        nc.gpsimd.tensor_scalar(
                            xb[:, kt, csl], xb[:, kt, csl],
                            aux_sb[:, kt : kt + 1], None, ALU.add,
                        )
                    nc.gpsimd.tensor_tensor(
                        xsqs[:, kt, :], xb[:, kt, csl], xb[:, kt, csl], ALU.mult
                    )
                slab_state[s] = xsqs

            def _slab_s2(s):
                csl = slice(s * SLAB, (s + 1) * SLAB)
                xsqs = slab_state[s]
                # pmu/psq share one PSUM bank -> single accumulation group
                stat = psA.tile([P, 512], f32, tag="a", name=f"stat{s}")
                for kt in range(KT):
                    nc.tensor.matmul(
                        stat[:, 0:SLAB], ones128, xb[:, kt, csl],
                        start=(kt == 0), stop=False, skip_group_check=True,
                    )
                    nc.tensor.matmul(
                        stat[:, SLAB : 2 * SLAB], ones128, xsqs[:, kt, :],
                        start=False, stop=(kt == KT - 1), skip_group_check=True,
                    )
                slab_state[s] = stat

            def _slab_s3(s):
                stat = slab_state[s]
                m1 = work.tile([P, SLAB], f32, tag="m1")
                nc.vector.tensor_scalar_mul(m1, stat[:, 0:SLAB], 1.0 / D)
                v1 = work.tile([P, SLAB], f32, tag="v1")
                nc.vector.tensor_scalar_mul(v1, stat[:, SLAB : 2 * SLAB], 1.0 / D)
                m2 = work.tile([P, SLAB], f32, tag="m2")
                nc.gpsimd.tensor_tensor(m2, m1, m1, ALU.mult)
                nc.gpsimd.tensor_tensor(v1, v1, m2, ALU.subtract)
                sd = work.tile([P, SLAB], f32, tag="sd")
                nc.scalar.activation(sd, v1, AF.Sqrt, bias=eps_col)
                r2b = work.tile([P, SLAB], f32, tag="r2b")
                nc.vector.reciprocal_approx_fast(r2b, sd)
                m1b = work.tile([P, SLAB], bf16, tag="m1b")
                nc.gpsimd.tensor_copy(m1b, m1)
                r2s = work.tile([P, SLAB], bf16, tag="r2s")
                nc.gpsimd.tensor_copy(r2s, r2b)
                slab_state[s] = (m1b, r2s)

            def _slab_s4(s):
                csl = slice(s * SLAB, (s + 1) * SLAB)
                m1b, r2s = slab_state.pop(s)
                for kt in range(KT):
                    nc.gpsimd.tensor_tensor(
                        x1aug[:, kt, csl], xb[:, kt, csl], m1b, ALU.subtract
                    )
                    nc.gpsimd.tensor_tensor(
                        x1aug[:, kt, csl], x1aug[:, kt, csl], r2s, ALU.mult
                    )

            # ============ phase B: attention ================================
            # Heads are processed in partition-tile pairs: the two heads of a
            # pair occupy partitions 0:64 / 64:128, so their K=64 logits
            # matmuls land in disjoint PE row groups and run concurrently
            # (row tiling).  AV matmuls lag L steps behind so the softmax exp
            # (split Act/DVE) is off the critical path.
            from collections import deque

            with tc.tile_pool(name="slabA", bufs=1) as slabpA, \
                 nc.named_scope("attn"):
                epi_q = deque()
                epi_bq = deque()
                proj_q = deque()
                pend = deque()
                L = 2

                def _flush_avq():
                    avq, vcol, et, kt = pend.popleft()
                    nc.tensor.matmul(
                        avq, vext[:, kt, vcol], et,
                        start=(kt == 0), stop=(kt == NKT - 1),
                    )

                def _epi_a(st):
                    pt, hp, qc, avq = st
                    rs_sb = attg.tile([1, 512], f32, tag="rsb", name="rs_sb",
                                      bufs=2)
                    nc.scalar.activation(rs_sb, avq[64:65, :], AF.Copy)
                    rc_f = attg.tile([1, 512], f32, tag="rcf", name="rcf",
                                     bufs=2)
                    nc.vector.reciprocal_approx_fast(rc_f, rs_sb)
                    rc_b = attg.tile([1, 512], bf16, tag="rcb", name="rcb",
                                     bufs=2)
                    nc.gpsimd.tensor_copy(rc_b, rc_f)
                    return (pt, hp, qc, avq, rc_b)

                def _epi_b(st):
                    pt, hp, qc, avq, rc_b = st
                    q0 = qc * 512
                    rbp = lgp.tile([P, 512], f32, tag="lg", name="rbp")[0:64, :]
                    nc.tensor.matmul(
                        rbp, ones128[0:1, 0:64], rc_b, start=True, stop=True
                    )
                    rbs = attg.tile([64, 512], bf16, tag="rbs", name="rbs",
                                    bufs=2)
                    nc.scalar.activation(rbs, rbp, AF.Copy)
                    nc.vector.tensor_tensor(
                        attnT[pt][hp * HD : (hp + 1) * HD, q0 : q0 + 512],
                        avq[0:64, :], rbs, ALU.mult,
                    )

                def _emit_proj():
                    qc, m = proj_q.popleft()
                    tsl = slice(qc * 512, (qc + 1) * 512)
                    ps = psA.tile([P, 512], f32, tag="a", name="projps")
                    for kt2 in range(2):
                        nc.tensor.matmul(
                            ps, wproj_sb[:, kt2, m * P : (m + 1) * P],
                            attnT[kt2][:, tsl], start=(kt2 == 0), stop=(kt2 == 1),
                        )
                    pb = poutp.tile([P, 512], bf16, tag="pout", name="pb")
                    nc.scalar.activation(pb, ps, AF.Copy)
                    # scatter: rank r's slab columns -> rows r*128.., col m*128..
                    nc.sync.dma_start(
                        partial_d[qc].rearrange(
                            "(r p) (m t) -> m p r t", r=4, m=KT
                        )[m],
                        pb.rearrange("p (r t) -> p r t", r=4),
                    )
                    if m == KT - 1:
                        with nc.named_scope("reducescatter"):
                            nc.gpsimd.collective_compute(
                                "ReduceScatter",
                                mybir.AluOpType.add,
                                replica_groups=groups,
                                ins=[partial_d[qc][:]],
                                outs=[x1p_d[qc][:]],
                            )

                for qc in range(NQC):
                    for pt in range(2):
                        q0 = qc * 512
                        avqs = [
                            avqp.tile([P, 512], f32, tag="avq",
                                      name=f"avq{qc}{pt}{hp}")[0:65, :]
                            for hp in range(2)
                        ]
                        for kt in range(NKT):
                            ksl = slice(kt * P, (kt + 1) * P)
                            for hp in range(2):
                                hsl = slice(hp * HD, (hp + 1) * HD)
                                lg = lgp.tile([P, 512], f32, tag="lg", name="lg")
                                nc.tensor.matmul(
                                    lg, qkvT[pt][hsl, 1, ksl],
                                    qkvT[pt][hsl, 0, q0 : q0 + 512],
                                    start=True, stop=True,
                                )
                                et = etp.tile([P, 512], bf16, tag="et")
                                if hp == 0 or kt % 8 == 7:
                                    nc.scalar.activation(
                                        et, lg, AF.Exp, scale=1.0 / np.sqrt(HD)
                                    )
                                else:
                                    nc.vector.tensor_scalar(
                                        et.bitcast(i16), lg, EXP_A, EXP_B,
                                        ALU.mult, ALU.add,
                                    )
                                vcol = slice(
                                    pt * 130 + hp * 65, pt * 130 + hp * 65 + 65
                                )
                                pend.append((avqs[hp], vcol, et, kt))
                            while len(pend) > 2 * L:
                                _flush_avq()
                            if kt in (0, 1) and epi_q:
                                epi_bq.append(_epi_a(epi_q.popleft()))
                            if kt in (3, 4) and epi_bq:
                                _epi_b(epi_bq.popleft())
                            if pt == 0 and kt in (5, 7, 9, 11, 13, 15) \
                                    and proj_q:
                                _emit_proj()
                            if pt == 1 and kt in (1, 3) and proj_q:
                                _emit_proj()
                            if qc >= 2:
                                s = qc - 2
                                if pt == 0 and kt == 4:
                                    _slab_s1(s, slabpA)
                                if pt == 1 and kt == 6:
                                    _slab_s2(s)
                                if pt == 1 and kt == 10:
                                    _slab_s3(s)
                                if pt == 1 and kt == 14:
                                    _slab_s4(s)
                        while pend:
                            _flush_avq()
                        for hp in range(2):
                            epi_q.append((pt, hp, qc, avqs[hp]))
                    proj_q.extend((qc, m) for m in range(8))

                while epi_q:
                    epi_bq.append(_epi_a(epi_q.popleft()))
                while epi_bq:
                    _epi_b(epi_bq.popleft())
                with nc.named_scope("proj"):
                    while proj_q:
                        _emit_proj()
                # slab 2: collective long done; runs during early MLP
                with nc.named_scope("x1_ln2"):
                    _slab_s1(2, slabpA)
                    _slab_s2(2)
                    _slab_s3(2)
                    _slab_s4(2)

        # ============ phase C + D: residual/LN2 per slab + 2-pass MLP =====
        w_stack = ExitStack()
        w2_pool = w_stack.enter_context(tc.tile_pool(name="w2pool", bufs=1))
        psB = w_stack.enter_context(tc.tile_pool(name="psB", bufs=1, space="PSUM"))
        H2S = 24  # h2 ring slots (down trails up by 16 f-tiles)
        h2T = w_stack.enter_context(tc.tile_pool(name="h2", bufs=1)).tile(
            [P, H2S, TC], bf16
        )
        slabB = w_stack.enter_context(tc.tile_pool(name="slabB", bufs=1))

        NF = FF // P  # 32 f-tiles
        NQ = NF // 4  # 8 f-tiles per weight quarter
        w2r = w2t_d.rearrange("(k p) d -> p k d", p=P)
        w2q = [None] * 4

        def _w2s(kt, m):
            return w2q[kt // NQ][:, kt % NQ, m * P : (m + 1) * P]

        assert not has_c2, "nonzero ln2_b not supported"
        HTC = TC // 2  # 256 tokens per MLP pass

        def _accs(sfx):
            acc4 = [
                psB.tile([P, 2 * HTC], f32, tag=f"acc{g}", name=f"m2{sfx}{g}")
                for g in range(4)
            ]
            return [
                acc4[m // 2][:, (m % 2) * HTC : (m % 2 + 1) * HTC]
                for m in range(KT)
            ]

        accs = _accs("p")
        with nc.named_scope("mlp"):
            for p_i in range(2):
                t0 = p_i * HTC
                tsl = slice(t0, t0 + HTC)
                if p_i == 1:
                    accs = _accs("q")
                for j in range(NF):
                    if p_i == 0 and j >= NQ and j % NQ == 0:
                        q = j // NQ - 1
                        w2q[q] = w2_pool.tile(
                            [P, NQ, D], bf16, tag=f"w2_{q}", name=f"w2q{q}"
                        )
                        nc.sync.dma_start(w2q[q], w2r[:, q * NQ : (q + 1) * NQ, :])
                    if p_i == 0 and j in (16, 18, 20, 22):
                        # slab 3's residual+LN2, gated on the last collective
                        with nc.named_scope("x1_ln2_s3"):
                            if j == 16:
                                _slab_s1(3, slabB)
                            elif j == 18:
                                _slab_s2(3)
                            elif j == 20:
                                _slab_s3(3)
                            else:
                                _slab_s4(3)
                    w1h = w1q[j // NQ]
                    msl = slice((j % NQ) * P, (j % NQ + 1) * P)
                    ps = psA.tile([P, HTC], f32, tag="a", name="m1ps")
                    for kt in range(KT):
                        nc.tensor.matmul(
                            ps, w1h[:, kt, msl], x1aug[:, kt, tsl],
                            start=(kt == 0), stop=(kt == KT - 1),
                        )
                    bias_arg = aux_sb[:, 8 + j : 9 + j] if has_b1 else 0.0
                    nc.scalar.activation(
                        h2T[:, j % H2S, tsl], ps, AF.Relu, bias=bias_arg
                    )
                    if j >= 2 * NQ:
                        kt2 = j - 2 * NQ
                        for m in range(KT):
                            nc.tensor.matmul(
                                accs[m], _w2s(kt2, m), h2T[:, kt2 % H2S, tsl],
                                start=(kt2 == 0 and m % 2 == 0), stop=False,
                                skip_group_check=True,
                            )
                if p_i == 0:
                    w2q[3] = w2_pool.tile([P, NQ, D], bf16, tag="w2_3", name="w2q3")
                    nc.sync.dma_start(w2q[3], w2r[:, 3 * NQ :, :])
                for kt2 in range(NF - 2 * NQ, NF):
                    for m in range(KT):
                        nc.tensor.matmul(
                            accs[m], _w2s(kt2, m), h2T[:, kt2 % H2S, tsl],
                            start=False,
                            stop=(kt2 == NF - 1 and m % 2 == 1),
                            skip_group_check=True,
                        )
                for m in range(KT):
                    ob = work.tile([P, HTC], f32, tag="ob", bufs=2)
                    nc.vector.tensor_tensor(ob, accs[m], xb[:, m, tsl], ALU.add)
                    if has_b2:
                        nc.vector.tensor_scalar(
                            ob, ob, aux_sb[:, 40 + m : 41 + m], None, ALU.add
                        )
                    nc.sync.dma_start(out_d[m * P : (m + 1) * P, tsl], ob)
        w_stack.close()

    nc.compile()
    return nc


def _slab_cols(c):
    """Column indices into xT [D, T] owned by core c, in kernel order."""
    bc, r = c // 4, c % 4
    cols = []
    for qc in range(NQC):
        base = bc * TB + qc * 512 + r * SLAB
        cols.append(np.arange(base, base + SLAB))
    return np.concatenate(cols)


def _prep_inputs(inputs):
    x = np.asarray(inputs["x"], np.float32)
    w_qkv = np.asarray(inputs["w_qkv"], np.float32)
    w_proj = np.asarray(inputs["w_proj"], np.float32)
    b_proj = np.asarray(inputs["b_proj"], np.float32)
    w1 = np.asarray(inputs["w1"], np.float32)
    b1 = np.asarray(inputs["b1"], np.float32)
    w2 = np.asarray(inputs["w2"], np.float32)
    b2 = np.asarray(inputs["b2"], np.float32)
    ln1_g = np.asarray(inputs["ln1_g"], np.float32)
    ln1_b = np.asarray(inputs["ln1_b"], np.float32)
    ln2_g = np.asarray(inputs["ln2_g"], np.float32)
    ln2_b = np.asarray(inputs["ln2_b"], np.float32)

    has_c1 = bool(np.any(ln1_b != 0))
    has_bproj = bool(np.any(b_proj != 0))
    has_c2 = bool(np.any(ln2_b != 0))
    has_b1 = bool(np.any(b1 != 0))
    has_b2 = bool(np.any(b2 != 0))
    flags = (has_c1, has_bproj, has_c2, has_b1, has_b2)

    xT = np.ascontiguousarray(x.reshape(T, D).T)  # [D, T] f32

    wg = w_qkv * ln1_g[None, :]  # [3D, D]
    Se = wg.sum(axis=1)  # [3D]
    Ce = w_qkv @ ln1_b  # [3D]
    w1g = w1 * ln2_g[None, :]  # [FF, D]
    C2 = w1 @ ln2_b
    if np.any(C2 != 0):
        raise NotImplementedError("nonzero ln2_b not supported")

    w1_aug = np.ascontiguousarray(w1g.T).astype(BF16)
    w2t = np.ascontiguousarray(w2.T).astype(BF16)  # [FF, D]

    in_maps = []
    for c in range(NCORES):
        bc, hg = c // 4, c % 4
        # batch-sliced augmented x
        x_aug = np.zeros((DAUG, TB), BF16)
        x_aug[:D] = xT[:, bc * TB : (bc + 1) * TB].astype(BF16)

        # qkv weights for 4 heads: two partition-tiles of head pairs
        wqkv_aug = np.zeros((DAUG, 6 * P), BF16)
        cstack = np.zeros((P, 6), np.float32)
        for pt in range(2):
            r0 = (4 * hg + 2 * pt) * HD  # 128 contiguous rows (2 heads)
            for m in range(3):
                rows = slice(m * D + r0, m * D + r0 + 2 * HD)
                csl = slice(pt * 3 * P + m * P, pt * 3 * P + (m + 1) * P)
                wqkv_aug[:D, csl] = wg[rows].T.astype(BF16)
                wqkv_aug[D, csl] = Se[rows].astype(BF16)
                cstack[:, pt * 3 + m] = Ce[rows]

        # proj rows for this core's 256 head dims
        wproj_c = np.ascontiguousarray(
            w_proj[:, 4 * hg * HD : (4 * hg + 4) * HD].T
        ).astype(BF16)  # [256, D]

        aux = np.zeros((P, 64), np.float32)
        aux[:, 0:8] = b_proj.reshape(KT, P).T
        aux[:, 8:40] = b1.reshape(FF // P, P).T
        aux[:, 40:48] = b2.reshape(KT, P).T
        aux[:, 48:54] = cstack

        in_maps.append(
            {
                "x_aug": x_aug,
                "x_c": np.ascontiguousarray(xT[:, _slab_cols(c)]).astype(BF16),
                "wqkv_aug": wqkv_aug,
                "wproj_c": wproj_c,
                "w1_aug": w1_aug,
                "w2t": w2t,
                "aux": aux,
            }
        )
    return flags, in_maps


def _run(inputs, trace=False, trace_kwargs=None):
    from concourse.bass_utils import run_bass_kernel_spmd

    flags, in_maps = _prep_inputs(inputs)
    if flags not in _CACHE:
        _CACHE[flags] = _build_program(*flags)
    nc = _CACHE[flags]
    res = run_bass_kernel_spmd(
        nc, in_maps, list(range(NCORES)), trace=trace,
        **(trace_kwargs or {}),
    )
    outT = np.empty((D, T), np.float32)
    for c in range(NCORES):
        outT[:, _slab_cols(c)] = res.results[c]["out_c"]
    out = np.ascontiguousarray(outT.T).reshape(B, S, D)
    return out, res


def kernel(**inputs):
    out, _ = _run(inputs, trace=False)
    return out


# revision 37
# speedup vs baseline: 1.1643x; 1.0266x over previous
"""Trainium2 Bass kernel for a pre-LN transformer block (B=2, S=2048, D=1024,
H=16, d_ff=4096), 8-way (batch, head-group) tensor-parallel:

- core c handles batch c//4 and heads 4*(c%4)..4*(c%4)+3: LN1+qkv run over the
  core's 2048 batch tokens only, attention over 4 heads
- softmax exp is split across engines: even key-tiles use the Activation
  engine's exact Exp, odd key-tiles use a Schraudolph-style int16 exponent
  construction on the DVE (bitcast to bf16)
- attention-proj partials are ReduceScattered per query-chunk (4 collectives),
  each fired as soon as that chunk's proj partials are done, so 3 of 4 overlap
  the remaining attention compute; each core owns four interleaved 128-token
  slabs (slab qc = tokens qc*512 + rank*128 ..+128) so the residual+LN2+MLP
  pipeline starts at attention end, with the MLP split into two 256-token
  passes (the second gated only on the last collective)
- token-sharded MLP with the full d_ff on each core (no second collective)

Activations live feature-major [feature, token].  LayerNorm is folded into the
matmuls via an augmented contraction row (-mu) and column (row-sums of the
g-scaled weights); the 1/sigma factor is applied on PSUM eviction.  Softmax is
computed unnormalized with a ones-column appended to V producing row sums, and
1/sum is applied on the attention-output eviction.
"""

import sys

for _p in ("/opt/trn_rl_repo",):
    if _p not in sys.path:
        sys.path.insert(0, _p)

import numpy as np
import ml_dtypes

B, S, D = 2, 2048, 1024
H, HD = 16, 64
FF = 4 * D
T = B * S  # 4096 tokens
NCORES = 8
TC = T // NCORES  # 512 tokens per core (MLP/out shard)
TB = S  # 2048 tokens per batch (per-core attention range)
P = 128
KT = D // P  # 8 k-tiles over D
KA = 9  # augmented k-tiles
DAUG = D + P  # 1152
EPS = 1e-5
NKT = TB // P  # 16 key tiles per batch
NQC = TB // 512  # 4 q-chunks of 512
SLAB = TC // NQC  # 128 tokens per owned slab
BF16 = ml_dtypes.bfloat16

# Schraudolph exp: bf16 bits ~= round(x*log2(e)*128 + (127*128 - 7.63))
LOG2E = float(np.log2(np.e))
EXP_A = 128.0 * LOG2E / np.sqrt(HD)  # logit scale 1/sqrt(HD) folded in
EXP_B = 127.0 * 128.0 - 7.63
# key tiles using exact Exp on the Activation engine (rest: Schraudolph on DVE)
SC_KT = frozenset({0, 2, 4, 6, 8, 10, 12, 14})

_CACHE = {}


def _build_program(has_c1, has_bproj, has_c2, has_b1, has_b2):
    import concourse.mybir as mybir
    import concourse.tile as tile
    from concourse import bacc
    from concourse.masks import make_identity
    from contextlib import ExitStack

    f32 = mybir.dt.float32
    bf16 = mybir.dt.bfloat16
    i16 = mybir.dt.int16
    AF = mybir.ActivationFunctionType
    ALU = mybir.AluOpType

    nc = bacc.Bacc(None, target_bir_lowering=False)

    # ---- I/O ----
    x_aug_d = nc.declare_dram_parameter("x_aug", [DAUG, TB], bf16, isOutput=False)
    x_c_d = nc.declare_dram_parameter("x_c", [D, TC], bf16, isOutput=False)
    wqkv_d = nc.declare_dram_parameter("wqkv_aug", [DAUG, 6 * P], bf16, isOutput=False)
    wproj_d = nc.declare_dram_parameter("wproj_c", [2 * P, D], bf16, isOutput=False)
    w1_d = nc.declare_dram_parameter("w1_aug", [D, FF], bf16, isOutput=False)
    w2t_d = nc.declare_dram_parameter("w2t", [FF, D], bf16, isOutput=False)
    aux_d = nc.declare_dram_parameter("aux", [P, 64], f32, isOutput=False)
    # aux columns: 0:8 -> b_proj as [128,8], 8:40 -> b1 as [128,32],
    # 40:48 -> b2 as [128,8], 48:54 -> C1 (qkv bias-fold) as [128,6]
    out_d = nc.declare_dram_parameter("out_c", [D, TC], f32, isOutput=True)

    groups = [[0, 1, 2, 3], [4, 5, 6, 7]]

    with tile.TileContext(nc) as tc, ExitStack() as ctx:
        const = ctx.enter_context(tc.tile_pool(name="const", bufs=1))
        dram = ctx.enter_context(tc.tile_pool(name="dram", bufs=1, space="DRAM"))

        ident = const.tile([P, P], bf16)
        make_identity(nc, ident)
        ones128 = const.tile([P, P], bf16)
        nc.any.memset(ones128, 1.0)
        eps_col = const.tile([P, 1], f32)
        nc.any.memset(eps_col, EPS)

        wqkv_sb = const.tile([P, KA, 6 * P], bf16)
        nc.sync.dma_start(wqkv_sb, wqkv_d.rearrange("(k p) e -> p k e", p=P))
        wproj_sb = const.tile([P, 2, D], bf16)
        nc.sync.dma_start(wproj_sb, wproj_d.rearrange("(k p) d -> p k d", p=P))
        aux_sb = const.tile([P, 64], f32)
        nc.sync.dma_start(aux_sb, aux_d[:])

        # long-lived activation tensors
        x1grp = ctx.enter_context(tc.tile_pool(name="x1grp", bufs=1))
        x1aug = x1grp.tile([P, KT, TC], bf16)
        work = ctx.enter_context(tc.tile_pool(name="work", bufs=1))

        psA = ctx.enter_context(tc.tile_pool(name="psA", bufs=2, space="PSUM"))

        # residual input, prefetched during attention
        resid = ctx.enter_context(tc.tile_pool(name="resid", bufs=1))
        xc = resid.tile([P, KT, TC], bf16, tag="xc")
        xb = resid.tile([P, KT, TC], bf16, tag="xb")

        # w1 weights, prefetched during attention
        w1_pool = ctx.enter_context(tc.tile_pool(name="w1pool", bufs=1))

        # proj partials per query chunk, wide-row layout for the collective:
        # row r*128 + p, col m*128 + t  <->  feature m*128+p, rank-r slab
        # token t (2KB rows so the ReduceScatter moves efficient lines)
        partial_d = [
            dram.tile([4 * P, KT * SLAB], bf16, tag=f"pp{qc}", name=f"pp{qc}")
            for qc in range(NQC)
        ]
        x1p_d = [
            dram.tile([P, KT * SLAB], bf16, tag=f"xp{qc}", name=f"xp{qc}")
            for qc in range(NQC)
        ]

        x_aug_r = x_aug_d.rearrange("(k p) t -> p k t", p=P)
        w1_noaug_r = w1_d.rearrange("(k p) f -> p k f", p=P)

        w1q = []
        with tc.tile_pool(name="qkvTp", bufs=1) as qkvT_pool, \
             tc.tile_pool(name="attnTp", bufs=1) as attnT_pool, \
             tc.tile_pool(name="attg", bufs=1) as attg, \
             tc.tile_pool(name="etp", bufs=9) as etp, \
             tc.tile_pool(name="poutp", bufs=3) as poutp, \
             tc.tile_pool(name="lgp", bufs=3, space="PSUM") as lgp, \
             tc.tile_pool(name="avqp", bufs=3, space="PSUM") as avqp:
            qkvT = [qkvT_pool.tile([P, 2, TB], bf16, name=f"qkvT{pt}") for pt in (0, 1)]
            attnT = [attnT_pool.tile([P, TB], bf16, name=f"attnT{pt}") for pt in (0, 1)]
            # vext: per key tile: [h0 | 1 | h1 | 1 | h2 | 1 | h3 | 1]
            vext = attg.tile([P, NKT, 4 * 65], bf16)

            # ============ phase A: LN1 stats + qkv + vext, per token chunk ===
            with tc.tile_pool(name="xaug", bufs=2) as xaug_pool, \
                 tc.tile_pool(name="workA", bufs=2) as workA, \
                 nc.named_scope("ln1_qkv"):
                for hp in range(4):
                    nc.any.memset(vext[:, :, hp * 65 + 64 : hp * 65 + 65], 1.0)
                for tch in range(NQC):
                    tsl = slice(tch * 512, (tch + 1) * 512)
                    xa = xaug_pool.tile([P, KA, 512], bf16, tag="xa")
                    nc.sync.dma_start(xa, x_aug_r[:, :, tsl])
                    pmu = psA.tile([P, 512], f32, tag="a", name="pmu")
                    psq = psA.tile([P, 512], f32, tag="a", name="psq")
                    for kt in range(KT):
                        xsq = workA.tile([P, 512], bf16, tag="xsq")
                        nc.vector.tensor_tensor(
                            xsq, xa[:, kt, :], xa[:, kt, :], ALU.mult
                        )
                        nc.tensor.matmul(
                            pmu, ones128, xa[:, kt, :],
                            start=(kt == 0), stop=(kt == KT - 1),
                        )
                        nc.tensor.matmul(
                            psq, ones128, xsq,
                            start=(kt == 0), stop=(kt == KT - 1),
                        )
                    m1 = workA.tile([P, 512], f32, tag="m1")
                    nc.vector.tensor_scalar_mul(m1, pmu, 1.0 / D)
                    # augmented row: -mu (bf16), partition 0 of k-tile 8
                    nc.vector.tensor_scalar_mul(xa[0:1, KT, :], m1[0:1, :], -1.0)
                    v1 = workA.tile([P, 512], f32, tag="v1")
                    nc.vector.tensor_scalar_mul(v1, psq, 1.0 / D)
                    m2 = workA.tile([P, 512], f32, tag="m2")
                    nc.vector.tensor_tensor(m2, m1, m1, ALU.mult)
                    nc.vector.tensor_tensor(v1, v1, m2, ALU.subtract)
                    sd = workA.tile([P, 512], f32, tag="sd")
                    nc.scalar.activation(sd, v1, AF.Sqrt, bias=eps_col)
                    r1b = xaug_pool.tile([P, 512], f32, tag="r1b")
                    nc.vector.reciprocal_approx_fast(r1b, sd)

                    for pt in range(2):
                        vtmp = None
                        # v first so its transposes can interleave behind the
                        # q/k matmul groups without stalling the chunk boundary
                        for m in (2, 0, 1):
                            msl = slice(pt * 3 * P + m * P, pt * 3 * P + (m + 1) * P)
                            ps = lgp.tile([P, 512], f32, tag="lg", name="qkvps")
                            for kt in range(KA):
                                nc.tensor.matmul(
                                    ps, wqkv_sb[:, kt, msl], xa[:, kt, :],
                                    start=(kt == 0), stop=(kt == KA - 1),
                                )
                            if m < 2:
                                dst = qkvT[pt][:, m, tsl]
                            else:
                                vtmp = etp.tile(
                                    [P, 512], bf16, tag="et", name=f"vtmp{pt}"
                                )
                                dst = vtmp
                            nc.vector.tensor_tensor(dst, ps, r1b, ALU.mult)
                            if has_c1:
                                nc.vector.tensor_scalar(
                                    dst, dst,
                                    aux_sb[:, 48 + pt * 3 + m : 49 + pt * 3 + m],
                                    None, ALU.add,
                                )
                        with nc.named_scope("vext"):
                            for k4 in range(4):
                                kt = tch * 4 + k4
                                pt_t = psA.tile([P, 512], bf16, tag="a", name="ptt")[
                                    :, 0:P
                                ]
                                nc.tensor.transpose(
                                    pt_t, vtmp[:, k4 * P : (k4 + 1) * P], ident
                                )
                                c0 = pt * 130
                                nc.vector.tensor_copy(
                                    vext[:, kt, c0 : c0 + 64], pt_t[:, 0:64]
                                )
                                nc.vector.tensor_copy(
                                    vext[:, kt, c0 + 65 : c0 + 129], pt_t[:, 64:128]
                                )

            # prefetch residual + MLP-up weights during attention
            nc.sync.dma_start(xc, x_c_d.rearrange("(k p) t -> p k t", p=P))
            FQ = FF // 4
            for q in range(4):
                w1qt = w1_pool.tile([P, KT, FQ], bf16, tag=f"w1_{q}", name=f"w1q{q}")
                nc.sync.dma_start(w1qt, w1_noaug_r[:, :, q * FQ : (q + 1) * FQ])
                w1q.append(w1qt)

            # ---- residual + LN2 pipeline, staged so it can interleave with
            # attention.  SBUF-only elementwise goes to the (idle) GpSimd
            # engine; PSUM reads stay on DVE/Scalar.
            slab_state = {}

            def _slab_s1(s, pool):
                csl = slice(s * SLAB, (s + 1) * SLAB)
                x1p = pool.tile([P, KT, SLAB], bf16, tag="x1p", name=f"x1p{s}")
                nc.sync.dma_start(
                    x1p, x1p_d[s].rearrange("p (k t) -> p k t", k=KT)
                )
                xsqs = pool.tile([P, KT, SLAB], bf16, tag="xsqs", name=f"xsqs{s}")
                for kt in range(KT):
                    nc.gpsimd.tensor_tensor(
                        xb[:, kt, csl], xc[:, kt, csl], x1p[:, kt, :], ALU.add
                    )
                    if has_bproj:
                        nc.gpsimd.tensor_scalar(
                            xb[:, kt, csl], xb[:, kt, csl],
                            aux_sb[:, kt : kt + 1], None, ALU.add,
                        )
                    nc.gpsimd.tensor_tensor(
                        xsqs[:, kt, :], xb[:, kt, csl], xb[:, kt, csl], ALU.mult
                    )
                slab_state[s] = xsqs

            def _slab_s2(s):
                csl = slice(s * SLAB, (s + 1) * SLAB)
                xsqs = slab_state[s]
                # pmu/psq share one PSUM bank -> single accumulation group
                stat = psA.tile([P, 512], f32, tag="a", name=f"stat{s}")
                for kt in range(KT):
                    nc.tensor.matmul(
                        stat[:, 0:SLAB], ones128, xb[:, kt, csl],
                        start=(kt == 0), stop=False, skip_group_check=True,
                    )
                    nc.tensor.matmul(
                        stat[:, SLAB : 2 * SLAB], ones128, xsqs[:, kt, :],
                        start=False, stop=(kt == KT - 1), skip_group_check=True,
                    )
                slab_state[s] = stat

            def _slab_s3(s):
                stat = slab_state[s]
                m1 = work.tile([P, SLAB], f32, tag="m1")
                nc.vector.tensor_scalar_mul(m1, stat[:, 0:SLAB], 1.0 / D)
                v1 = work.tile([P, SLAB], f32, tag="v1")
                nc.vector.tensor_scalar_mul(v1, stat[:, SLAB : 2 * SLAB], 1.0 / D)
                m2 = work.tile([P, SLAB], f32, tag="m2")
                nc.gpsimd.tensor_tensor(m2, m1, m1, ALU.mult)
                nc.gpsimd.tensor_tensor(v1, v1, m2, ALU.subtract)
                sd = work.tile([P, SLAB], f32, tag="sd")
                nc.scalar.activation(sd, v1, AF.Sqrt, bias=eps_col)
                r2b = work.tile([P, SLAB], f32, tag="r2b")
                nc.vector.reciprocal_approx_fast(r2b, sd)
                m1b = work.tile([P, SLAB], bf16, tag="m1b")
                nc.gpsimd.tensor_copy(m1b, m1)
                r2s = work.tile([P, SLAB], bf16, tag="r2s")
                nc.gpsimd.tensor_copy(r2s, r2b)
                slab_state[s] = (m1b, r2s)

            def _slab_s4(s):
                csl = slice(s * SLAB, (s + 1) * SLAB)
                m1b, r2s = slab_state.pop(s)
                for kt in range(KT):
                    nc.gpsimd.tensor_tensor(
                        x1aug[:, kt, csl], xb[:, kt, csl], m1b, ALU.subtract
                    )
                    nc.gpsimd.tensor_tensor(
                        x1aug[:, kt, csl], x1aug[:, kt, csl], r2s, ALU.mult
                    )

            # ============ phase B: attention ================================
            # Heads are processed in partition-tile pairs: the two heads of a
            # pair occupy partitions 0:64 / 64:128, so their K=64 logits
            # matmuls land in disjoint PE row groups and run concurrently
            # (row tiling).  AV matmuls lag L steps behind so the softmax exp
            # (split Act/DVE) is off the critical path.
            from collections import deque

            with tc.tile_pool(name="slabA", bufs=1) as slabpA, \
                 nc.named_scope("attn"):
                epi_q = deque()
                epi_bq = deque()
                proj_q = deque()
                pend = deque()
                L = 3

                def _flush_avq():
                    avq, vcol, et, kt = pend.popleft()
                    nc.tensor.matmul(
                        avq, vext[:, kt, vcol], et,
                        start=(kt == 0), stop=(kt == NKT - 1),
                    )

                def _epi_a(st):
                    pt, hp, qc, avq = st
                    rs_sb = attg.tile([1, 512], f32, tag="rsb", name="rs_sb",
                                      bufs=2)
                    nc.scalar.activation(rs_sb, avq[64:65, :], AF.Copy)
                    rc_f = attg.tile([1, 512], f32, tag="rcf", name="rcf",
                                     bufs=2)
                    nc.vector.reciprocal_approx_fast(rc_f, rs_sb)
                    rc_b = attg.tile([1, 512], bf16, tag="rcb", name="rcb",
                                     bufs=2)
                    nc.gpsimd.tensor_copy(rc_b, rc_f)
                    return (pt, hp, qc, avq, rc_b)

                def _epi_b(st):
                    pt, hp, qc, avq, rc_b = st
                    q0 = qc * 512
                    rbp = lgp.tile([P, 512], f32, tag="lg", name="rbp")[0:64, :]
                    nc.tensor.matmul(
                        rbp, ones128[0:1, 0:64], rc_b, start=True, stop=True
                    )
                    rbs = attg.tile([64, 512], bf16, tag="rbs", name="rbs",
                                    bufs=2)
                    nc.scalar.activation(rbs, rbp, AF.Copy)
                    nc.vector.tensor_tensor(
                        attnT[pt][hp * HD : (hp + 1) * HD, q0 : q0 + 512],
                        avq[0:64, :], rbs, ALU.mult,
                    )

                def _emit_proj():
                    qc, m = proj_q.popleft()
                    tsl = slice(qc * 512, (qc + 1) * 512)
                    ps = psA.tile([P, 512], f32, tag="a", name="projps")
                    for kt2 in range(2):
                        nc.tensor.matmul(
                            ps, wproj_sb[:, kt2, m * P : (m + 1) * P],
                            attnT[kt2][:, tsl], start=(kt2 == 0), stop=(kt2 == 1),
                        )
                    pb = poutp.tile([P, 512], bf16, tag="pout", name="pb")
                    nc.scalar.activation(pb, ps, AF.Copy)
                    # scatter: rank r's slab columns -> rows r*128.., col m*128..
                    nc.sync.dma_start(
                        partial_d[qc].rearrange(
                            "(r p) (m t) -> m p r t", r=4, m=KT
                        )[m],
                        pb.rearrange("p (r t) -> p r t", r=4),
                    )
                    if m == KT - 1:
                        with nc.named_scope("reducescatter"):
                            nc.gpsimd.collective_compute(
                                "ReduceScatter",
                                mybir.AluOpType.add,
                                replica_groups=groups,
                                ins=[partial_d[qc][:]],
                                outs=[x1p_d[qc][:]],
                            )

                for qc in range(NQC):
                    for pt in range(2):
                        q0 = qc * 512
                        avqs = [
                            avqp.tile([P, 512], f32, tag="avq",
                                      name=f"avq{qc}{pt}{hp}")[0:65, :]
                            for hp in range(2)
                        ]
                        for kt in range(NKT):
                            ksl = slice(kt * P, (kt + 1) * P)
                            for hp in range(2):
                                hsl = slice(hp * HD, (hp + 1) * HD)
                                lg = lgp.tile([P, 512], f32, tag="lg", name="lg")
                                nc.tensor.matmul(
                                    lg, qkvT[pt][hsl, 1, ksl],
                                    qkvT[pt][hsl, 0, q0 : q0 + 512],
                                    start=True, stop=True,
                                )
                                et = etp.tile([P, 512], bf16, tag="et")
                                if hp == 0 or kt % 8 == 7:
                                    nc.scalar.activation(
                                        et, lg, AF.Exp, scale=1.0 / np.sqrt(HD)
                                    )
                                else:
                                    nc.vector.tensor_scalar(
                                        et.bitcast(i16), lg, EXP_A, EXP_B,
                                        ALU.mult, ALU.add,
                                    )
                                vcol = slice(
                                    pt * 130 + hp * 65, pt * 130 + hp * 65 + 65
                                )
                                pend.append((avqs[hp], vcol, et, kt))
                            while len(pend) > 2 * L:
                                _flush_avq()
                            if kt in (0, 1) and epi_q:
                                epi_bq.append(_epi_a(epi_q.popleft()))
                            if kt in (3, 4) and epi_bq:
                                _epi_b(epi_bq.popleft())
                            if pt == 0 and kt in (5, 7, 9, 11, 13, 15) \
                                    and proj_q:
                                _emit_proj()
                            if pt == 1 and kt in (1, 3) and proj_q:
                                _emit_proj()
                            if qc >= 2:
                                s = qc - 2
                                if pt == 0 and kt == 4:
                                    _slab_s1(s, slabpA)
                                if pt == 1 and kt == 6:
                                    _slab_s2(s)
                                if pt == 1 and kt == 10:
                                    _slab_s3(s)
                                if pt == 1 and kt == 14:
                                    _slab_s4(s)
                        while pend:
                            _flush_avq()
                        for hp in range(2):
                            epi_q.append((pt, hp, qc, avqs[hp]))
                    proj_q.extend((qc, m) for m in range(8))

                while epi_q:
                    epi_bq.append(_epi_a(epi_q.popleft()))
                while epi_bq:
                    _epi_b(epi_bq.popleft())
                with nc.named_scope("proj"):
                    while proj_q:
                        _emit_proj()
                # slab 2: collective long done; runs during early MLP
                with nc.named_scope("x1_ln2"):
                    _slab_s1(2, slabpA)
                    _slab_s2(2)
                    _slab_s3(2)
                    _slab_s4(2)

        # ============ phase C + D: residual/LN2 per slab + 2-pass MLP =====
        w_stack = ExitStack()
        w2_pool = w_stack.enter_context(tc.tile_pool(name="w2pool", bufs=1))
        psB = w_stack.enter_context(tc.tile_pool(name="psB", bufs=1, space="PSUM"))
        H2S = 24  # h2 ring slots (down trails up by 16 f-tiles)
        h2T = w_stack.enter_context(tc.tile_pool(name="h2", bufs=1)).tile(
            [P, H2S, TC], bf16
        )
        slabB = w_stack.enter_context(tc.tile_pool(name="slabB", bufs=1))

        NF = FF // P  # 32 f-tiles
        NQ = NF // 4  # 8 f-tiles per weight quarter
        w2r = w2t_d.rearrange("(k p) d -> p k d", p=P)
        w2q = [None] * 4

        def _w2s(kt, m):
            return w2q[kt // NQ][:, kt % NQ, m * P : (m + 1) * P]

        assert not has_c2, "nonzero ln2_b not supported"
        HTC = TC // 2  # 256 tokens per MLP pass

        def _accs(sfx):
            acc4 = [
                psB.tile([P, 2 * HTC], f32, tag=f"acc{g}", name=f"m2{sfx}{g}")
                for g in range(4)
            ]
            return [
                acc4[m // 2][:, (m % 2) * HTC : (m % 2 + 1) * HTC]
                for m in range(KT)
            ]

        accs = _accs("p")
        with nc.named_scope("mlp"):
            for p_i in range(2):
                t0 = p_i * HTC
                tsl = slice(t0, t0 + HTC)
                if p_i == 1:
                    accs = _accs("q")
                for j in range(NF):
                    if p_i == 0 and j >= NQ and j % NQ == 0:
                        q = j // NQ - 1
                        w2q[q] = w2_pool.tile(
                            [P, NQ, D], bf16, tag=f"w2_{q}", name=f"w2q{q}"
                        )
                        nc.sync.dma_start(w2q[q], w2r[:, q * NQ : (q + 1) * NQ, :])
                    if p_i == 0 and j in (24, 26, 28, 30):
                        # slab 3's residual+LN2, gated on the last collective
                        with nc.named_scope("x1_ln2_s3"):
                            if j == 24:
                                _slab_s1(3, slabB)
                            elif j == 26:
                                _slab_s2(3)
                            elif j == 28:
                                _slab_s3(3)
                            else:
                                _slab_s4(3)
                    w1h = w1q[j // NQ]
                    msl = slice((j % NQ) * P, (j % NQ + 1) * P)
                    ps = psA.tile([P, HTC], f32, tag="a", name="m1ps")
                    for kt in range(KT):
                        nc.tensor.matmul(
                            ps, w1h[:, kt, msl], x1aug[:, kt, tsl],
                            start=(kt == 0), stop=(kt == KT - 1),
                        )
                    bias_arg = aux_sb[:, 8 + j : 9 + j] if has_b1 else 0.0
                    nc.scalar.activation(
                        h2T[:, j % H2S, tsl], ps, AF.Relu, bias=bias_arg
                    )
                    if j >= 2 * NQ:
                        kt2 = j - 2 * NQ
                        for m in range(KT):
                            nc.tensor.matmul(
                                accs[m], _w2s(kt2, m), h2T[:, kt2 % H2S, tsl],
                                start=(kt2 == 0 and m % 2 == 0), stop=False,
                                skip_group_check=True,
                            )
                if p_i == 0:
                    w2q[3] = w2_pool.tile([P, NQ, D], bf16, tag="w2_3", name="w2q3")
                    nc.sync.dma_start(w2q[3], w2r[:, 3 * NQ :, :])
                for kt2 in range(NF - 2 * NQ, NF):
                    for m in range(KT):
                        nc.tensor.matmul(
                            accs[m], _w2s(kt2, m), h2T[:, kt2 % H2S, tsl],
                            start=False,
                            stop=(kt2 == NF - 1 and m % 2 == 1),
                            skip_group_check=True,
                        )
                for m in range(KT):
                    ob = work.tile([P, HTC], f32, tag="ob", bufs=2)
                    nc.vector.tensor_tensor(ob, accs[m], xb[:, m, tsl], ALU.add)
                    if has_b2:
                        nc.vector.tensor_scalar(
                            ob, ob, aux_sb[:, 40 + m : 41 + m], None, ALU.add
                        )
                    nc.sync.dma_start(out_d[m * P : (m + 1) * P, tsl], ob)
        w_stack.close()

    nc.compile()
    return nc


def _slab_cols(c):
    """Column indices into xT [D, T] owned by core c, in kernel order."""
    bc, r = c // 4, c % 4
    cols = []
    for qc in range(NQC):
        base = bc * TB + qc * 512 + r * SLAB
        cols.append(np.arange(base, base + SLAB))
    return np.concatenate(cols)


def _prep_inputs(inputs):
    x = np.asarray(inputs["x"], np.float32)
    w_qkv = np.asarray(inputs["w_qkv"], np.float32)
    w_proj = np.asarray(inputs["w_proj"], np.float32)
    b_proj = np.asarray(inputs["b_proj"], np.float32)
    w1 = np.asarray(inputs["w1"], np.float32)
    b1 = np.asarray(inputs["b1"], np.float32)
    w2 = np.asarray(inputs["w2"], np.float32)
    b2 = np.asarray(inputs["b2"], np.float32)
    ln1_g = np.asarray(inputs["ln1_g"], np.float32)
    ln1_b = np.asarray(inputs["ln1_b"], np.float32)
    ln2_g = np.asarray(inputs["ln2_g"], np.float32)
    ln2_b = np.asarray(inputs["ln2_b"], np.float32)

    has_c1 = bool(np.any(ln1_b != 0))
    has_bproj = bool(np.any(b_proj != 0))
    has_c2 = bool(np.any(ln2_b != 0))
    has_b1 = bool(np.any(b1 != 0))
    has_b2 = bool(np.any(b2 != 0))
    flags = (has_c1, has_bproj, has_c2, has_b1, has_b2)

    xT = np.ascontiguousarray(x.reshape(T, D).T)  # [D, T] f32

    wg = w_qkv * ln1_g[None, :]  # [3D, D]
    Se = wg.sum(axis=1)  # [3D]
    Ce = w_qkv @ ln1_b  # [3D]
    w1g = w1 * ln2_g[None, :]  # [FF, D]
    C2 = w1 @ ln2_b
    if np.any(C2 != 0):
        raise NotImplementedError("nonzero ln2_b not supported")

    w1_aug = np.ascontiguousarray(w1g.T).astype(BF16)
    w2t = np.ascontiguousarray(w2.T).astype(BF16)  # [FF, D]

    in_maps = []
    for c in range(NCORES):
        bc, hg = c // 4, c % 4
        # batch-sliced augmented x
        x_aug = np.zeros((DAUG, TB), BF16)
        x_aug[:D] = xT[:, bc * TB : (bc + 1) * TB].astype(BF16)

        # qkv weights for 4 heads: two partition-tiles of head pairs
        wqkv_aug = np.zeros((DAUG, 6 * P), BF16)
        cstack = np.zeros((P, 6), np.float32)
        for pt in range(2):
            r0 = (4 * hg + 2 * pt) * HD  # 128 contiguous rows (2 heads)
            for m in range(3):
                rows = slice(m * D + r0, m * D + r0 + 2 * HD)
                csl = slice(pt * 3 * P + m * P, pt * 3 * P + (m + 1) * P)
                wqkv_aug[:D, csl] = wg[rows].T.astype(BF16)
                wqkv_aug[D, csl] = Se[rows].astype(BF16)
                cstack[:, pt * 3 + m] = Ce[rows]

        # proj rows for this core's 256 head dims
        wproj_c = np.ascontiguousarray(
            w_proj[:, 4 * hg * HD : (4 * hg + 4) * HD].T
        ).astype(BF16)  # [256, D]

        aux = np.zeros((P, 64), np.float32)
        aux[:, 0:8] = b_proj.reshape(KT, P).T
        aux[:, 8:40] = b1.reshape(FF // P, P).T
        aux[:, 40:48] = b2.reshape(KT, P).T
        aux[:, 48:54] = cstack

        in_maps.append(
            {
                "x_aug": x_aug,
                "x_c": np.ascontiguousarray(xT[:, _slab_cols(c)]).astype(BF16),
                "wqkv_aug": wqkv_aug,
                "wproj_c": wproj_c,
                "w1_aug": w1_aug,
                "w2t": w2t,
                "aux": aux,
            }
        )
    return flags, in_maps


def _run(inputs, trace=False, trace_kwargs=None):
    from concourse.bass_utils import run_bass_kernel_spmd

    flags, in_maps = _prep_inputs(inputs)
    if flags not in _CACHE:
        _CACHE[flags] = _build_program(*flags)
    nc = _CACHE[flags]
    res = run_bass_kernel_spmd(
        nc, in_maps, list(range(NCORES)), trace=trace,
        **(trace_kwargs or {}),
    )
    outT = np.empty((D, T), np.float32)
    for c in range(NCORES):
        outT[:, _slab_cols(c)] = res.results[c]["out_c"]
    out = np.ascontiguousarray(outT.T).reshape(B, S, D)
    return out, res


def kernel(**inputs):
    out, _ = _run(inputs, trace=False)
    return out
